# revision 1
# baseline (speedup 1.0000x reference)
"""Trainium2 Bass kernel for nn_DynConv2d (DGCNN EdgeConv layer).

Reference computation (B=2, C=64, N=8192, K=16, C_out=64):
  f = x[:,:,:,0]
  nn_idx = top-16 nearest neighbors by squared L2 over point features
  feat = concat([x_i, x_j - x_i])          # (B, 2C, N, K)
  y = W @ feat                             # 1x1 conv
  y = BatchNorm2d(y)  (training stats over (B,N,K))
  y = LeakyReLU(0.2)(y)
  out = max over K                         # (B, C_out, N)

Algebraic restructuring:
  * W @ [x_i; x_j - x_i] = u[:,i] + v[:,j] with u = (W1-W2)@f, v = W2@f.
  * BN+LeakyReLU is per-channel monotone, so max over K commutes; the kernel
    returns max_k(u+v_j) plus the BN batch stats (sum / sum-sq); the final
    affine + lrelu runs on host.
  * KNN score s = q.m - |m|^2/2 via a 128-contraction fp16 matmul: rows 0:64
    are features, rows 64:128 of keys hold -f^2/2 (q side holds ones), so no
    separate |m|^2 row materialization is needed.

Top-16 selection (per query row of 8192 fp32 PSUM scores):
  * Scalar engine evacuates PSUM -> SBUF converting to fp16 *into the high
    u16 lane* of a packed fp32 word whose low u16 lane is the key index
    (preloaded from a host template).  Numeric fp32 ordering of the packed
    word == lexicographic (fp16 score, index) ordering, so a single DVE
    Max top-8 per 1024-chunk yields values *and* indices in one pass - no
    MaxIndex, no scatter.
  * A per-query bias (-(q.mbar - c0 + 3.9|q|)) is added during evacuation to
    center the interesting (top-16) scores near zero, which shrinks the fp16
    rounding error where it matters.  Any per-query constant preserves the
    within-row order, so this never breaks correctness.
  * L2: Max + MatchReplace + Max over the 64 chunk-candidates -> top-16
    packed words; low lanes are the global key indices.
  * Indices are relayed out to the dma_gather wrap layout (16 partitions,
    idx[n%16, n//16] = nn-index of slot n = k*128+q) with 16 tiny 16x16 PE
    transposes, then one InstDMAGatherAnt fetches all 2048 neighbor rows of
    v^T per query tile.

Sharding: 8 cores; core c handles batch c//4, query block c%4 (2048 queries),
against all 8192 keys of its batch.
"""

import os
import sys

import numpy as np

sys.path.insert(0, "/opt/trn_rl_repo")

import concourse.bacc as bacc
import concourse.bass as bass
import concourse.mybir as mybir
import concourse.tile as tile
from concourse.masks import make_identity

F32 = mybir.dt.float32
F16 = mybir.dt.float16
U16 = mybir.dt.uint16
I16 = mybir.dt.int16

BN_EPS = 1e-5
LRELU_SLOPE = 0.2
BIAS_KAPPA = 3.9


def default_cfg():
    return dict(C=64, NK=8192, NQ=2048, K=16, CH=1024)


def emit(tc, ins, outs, cfg, uniq=""):
    """Per-core program.

    ins:  f (64, NK) f16, fq (64, NQ) f16 (this core's query slice),
          w2t (C, C) f16, wat (C, C) f16,
          mb (128, 1) f16  [rows 0:64 = -mean_keys(f), rows 64:128 = c0/64],
          pkt (128, NK) f32 [u32 words: low u16 = column index, high = 0]
    outs: out_o (NQ, C) f16   max-over-K of u+v (pre-BN), query-major
          out_s (128, 512) f32  rows {0,32,64,96} = psum stats
    """
    nc = tc.nc
    C = cfg["C"]          # 64
    NK = cfg["NK"]        # 8192
    NQ = cfg["NQ"]        # 2048
    K = cfg["K"]          # 16
    CH = cfg["CH"]        # 1024  L1 top-8 chunk
    NQT = NQ // 128       # 16 query tiles
    NCH = NK // CH        # 8 chunks
    CAND = 8 * NCH        # 64 candidates
    H = K * C // 2        # 512 (stats half-width)

    f, fq, w2t, wat, mb, pkt = (ins["f"], ins["fq"], ins["w2t"], ins["wat"],
                                ins["mb"], ins["pkt"])
    out_o, out_s = outs["out_o"], outs["out_s"]

    from contextlib import ExitStack
    ctx = ExitStack()
    dram_pool = ctx.enter_context(tc.tile_pool(name="dram" + uniq, bufs=1,
                                               space="DRAM"))
    vt_dram = dram_pool.tile([NK, C], F32, tag="vt", name="vt_t" + uniq)[:]

    cpool = ctx.enter_context(tc.tile_pool(name="consts" + uniq, bufs=1))
    big = ctx.enter_context(tc.tile_pool(name="big" + uniq, bufs=1))
    tk = ctx.enter_context(tc.tile_pool(name="topk" + uniq, bufs=2))
    vpool = ctx.enter_context(tc.tile_pool(name="vstage" + uniq, bufs=8))
    gpool = ctx.enter_context(tc.tile_pool(name="gather" + uniq, bufs=3))
    tpool = ctx.enter_context(tc.tile_pool(name="tail" + uniq, bufs=2))
    ps_score = ctx.enter_context(tc.tile_pool(name="ps_score" + uniq, bufs=2,
                                              space="PSUM"))
    ps_stat = ctx.enter_context(tc.tile_pool(name="ps_stat" + uniq, bufs=1,
                                             space="PSUM"))
    ps_misc = ctx.enter_context(tc.tile_pool(name="ps_misc" + uniq, bufs=3,
                                             space="PSUM"))

    # ---- constants / inputs ----
    identity = cpool.tile([128, 128], F32, tag="ident")
    make_identity(nc, identity[:])
    ones64 = cpool.tile([64, 1], F16, tag="ones64")
    nc.vector.memset(ones64[:], 1.0)
    ones128 = cpool.tile([128, 1], F16, tag="ones128")
    nc.vector.memset(ones128[:], 1.0)
    # w2t lives at base partition 64 to pair with keys_aug[64:128] in matmuls
    w2t_sb = cpool.tile([128, C], F16, tag="w2t")
    nc.sync.dma_start(out=w2t_sb[64:128, :], in_=w2t)
    wat_sb = cpool.tile([C, C], F16, tag="wat")
    nc.sync.dma_start(out=wat_sb[:], in_=wat)
    mb_sb = cpool.tile([128, 1], F16, tag="mb")
    nc.sync.dma_start(out=mb_sb[:], in_=mb)

    # keys_aug: rows 0:64 = f, rows 64:128 = -f^2/2 (built in place from a
    # second copy of f so every engine op stays partition-aligned).
    # DMAs are spread across engine queues so the prologue isn't serialized
    # on SP.
    # Prologue is scheduled at quarter granularity so the first score matmul
    # (which needs keys_aug rows 0:128 incl. -f^2/2) can start ~4us in:
    #   SP queue:  w/mb, q_aug, keys1 quarters, pkt0 chunk-pieces, keys1 rest
    #   Act queue: keys2 quarters (feeds vt matmuls + f^2)
    #   Pool:      f^2 quarters (after the vt matmuls that read raw f), pkt1
    #   DVE:       q2 square, vt/ut psum->sbuf copies
    keys_aug = big.tile([128, NK], F16, tag="keys_aug")
    q_aug = big.tile([128, NQ], F16, tag="q_aug")
    nc.sync.dma_start(out=q_aug[0:64, :], in_=fq)
    # rows 64:128 hold -0.5 so the score matmul contracts -0.5 * f^2 rows
    nc.gpsimd.memset(q_aug[64:128, :], -0.5)
    NQR = NK // 4
    for r in range(4):
        nc.scalar.dma_start(out=keys_aug[64:128, bass.ts(r, NQR)],
                            in_=f[:, bass.ts(r, NQR)])

    # fq^2 for the per-query |q| bias estimate (DVE, 2-byte fast path)
    q2 = big.tile([64, NQ], F16, tag="q2")
    nc.vector.tensor_tensor(out=q2[:], in0=q_aug[0:64, :], in1=q_aug[0:64, :],
                            op=mybir.AluOpType.mult)

    # Single packed score buffer (low u16 lanes = key index template).  The
    # evac(t+1, c) -> Max(t, c) WAR is 8 chunk-slots stale, so one buffer
    # pipelines with no stalls.  keys1 quarters and template chunks
    # interleave on SP so chunk c's template lands just before Max(0, c).
    packed = big.tile([128, NK], F32, tag="packed")
    for r in range(4):
        nc.sync.dma_start(out=keys_aug[0:64, bass.ts(r, NQR)],
                          in_=f[:, bass.ts(r, NQR)])
        if r < 2:
            for c in (2 * r, 2 * r + 1):
                nc.sync.dma_start(out=packed[:, c * CH:(c + 1) * CH],
                                  in_=pkt[:, c * CH:(c + 1) * CH])
    # gather index tile (rows 16:128 must stay 0 for the executor's bounds
    # check; only rows 0:16 are consumed)
    gtidx = big.tile([128, 128], I16, tag="gtidx")
    nc.gpsimd.memset(gtidx[:], 0)

    # ---- v^T matmuls (read raw f in rows 64:128) + quartered -f^2/2 ----
    # Emission order per quarter: vt matmuls reading that quarter first, then
    # the in-place squaring of the same columns (readers before writer).
    vt_view = vt_dram.rearrange("(t c p) ch -> t p c ch", t=8, c=8)
    for t in range(8):
        ps_vt = ps_misc.tile([128, 512], F32, tag="misc")
        for c in range(8):
            nc.tensor.matmul(ps_vt[:, c * C:(c + 1) * C],
                             lhsT=keys_aug[64:128, bass.ts(8 * t + c, 128)],
                             rhs=w2t_sb[64:128, :], start=True, stop=True)
        vt_sb = vpool.tile([128, 512], F32, tag="vt_sb")
        nc.vector.tensor_copy(out=vt_sb[:], in_=ps_vt[:])
        nc.sync.dma_start(out=vt_view[t],
                          in_=vt_sb[:].rearrange("p (c ch) -> p c ch", c=8))
        if t % 2 == 1:
            r = t // 2
            nc.gpsimd.tensor_tensor(
                out=keys_aug[64:128, bass.ts(r, NQR)],
                in0=keys_aug[64:128, bass.ts(r, NQR)],
                in1=keys_aug[64:128, bass.ts(r, NQR)],
                op=mybir.AluOpType.mult)
    # late template chunks ride the Pool DMA queue (deadline: Max(0, c))
    for c in range(4, NCH):
        nc.gpsimd.dma_start(out=packed[:, c * CH:(c + 1) * CH],
                            in_=pkt[:, c * CH:(c + 1) * CH])

    # per-query bias for all tiles: bias_all[:, t] = -(q.mbar - c0 + kappa*|q|)
    ps_ball = ps_misc.tile([128, 512], F32, tag="misc", name="psball" + uniq)
    for qt in range(NQT):
        nc.tensor.matmul(ps_ball[:, qt:qt + 1], lhsT=q_aug[:, bass.ts(qt, 128)],
                         rhs=mb_sb[:], start=True, stop=True)
        nc.tensor.matmul(ps_ball[:, NQT + qt:NQT + qt + 1],
                         lhsT=q2[:, bass.ts(qt, 128)], rhs=ones64[:],
                         start=True, stop=True)
    qn_all = big.tile([128, NQT], F32, tag="qn_all")
    nc.scalar.activation(qn_all[:], ps_ball[:, NQT:2 * NQT],
                         mybir.ActivationFunctionType.Sqrt, scale=1.0)
    bias_all = big.tile([128, NQT], F32, tag="bias_all")
    nc.vector.scalar_tensor_tensor(
        out=bias_all[:], in0=qn_all[:], scalar=-BIAS_KAPPA,
        in1=ps_ball[:, 0:NQT],
        op0=mybir.AluOpType.mult, op1=mybir.AluOpType.add)

    ut_sb = big.tile([128, NQT * C], F32, tag="ut")
    for h in range(2):
        ps_ut = ps_misc.tile([128, 512], F32, tag="misc")
        for j in range(8):
            nc.tensor.matmul(ps_ut[:, j * C:(j + 1) * C],
                             lhsT=q_aug[0:64, bass.ts(8 * h + j, 128)],
                             rhs=wat_sb[:], start=True, stop=True)
        nc.vector.tensor_copy(out=ut_sb[:, h * 512:(h + 1) * 512], in_=ps_ut[:])

    stats_ps = ps_stat.tile([128, 512], F32, tag="stat")

    # ---- per-tile tail: A = G + u, stats, max over K, out DMA ----
    def tail(tq):
        G = G_tiles.pop(tq)
        ut_qt = ut_sb[:, bass.ts(tq, C)]
        A = tpool.tile([128, K * C], F16, tag="A")
        nc.gpsimd.tensor_tensor(
            out=A[:].rearrange("p (k o) -> p k o", k=K),
            in0=G[:].rearrange("p (k o) -> p k o", k=K),
            in1=ut_qt.unsqueeze(1).broadcast_to([128, K, C]),
            op=mybir.AluOpType.add)
        A2 = tpool.tile([128, K * C], F16, tag="A2")
        nc.gpsimd.tensor_tensor(out=A2[:], in0=A[:], in1=A[:],
                                op=mybir.AluOpType.mult)
        st, sp = (tq == 0), (tq == NQT - 1)
        nc.tensor.matmul(stats_ps[0:1, 0:H], lhsT=ones128[:], rhs=A[:, 0:H],
                         start=st, stop=sp, tile_position=(0, 0))
        nc.tensor.matmul(stats_ps[32:33, 0:H], lhsT=ones128[:], rhs=A[:, H:],
                         start=st, stop=sp, tile_position=(0, 32))
        nc.tensor.matmul(stats_ps[64:65, 0:H], lhsT=ones128[:], rhs=A2[:, 0:H],
                         start=st, stop=sp, tile_position=(0, 64))
        nc.tensor.matmul(stats_ps[96:97, 0:H], lhsT=ones128[:], rhs=A2[:, H:],
                         start=st, stop=sp, tile_position=(0, 96))
        # max over K: fold tree (8,4,2,1) on DVE fp16 (2x path; Pool has no
        # ISA max, and contiguous halves keep the 2-byte fast mode)
        M1 = tpool.tile([128, 8 * C], F16, tag="M1")
        nc.vector.tensor_tensor(out=M1[:], in0=A[:, 0:8 * C], in1=A[:, 8 * C:],
                                op=mybir.AluOpType.max)
        M2 = tpool.tile([128, 4 * C], F16, tag="M2")
        nc.vector.tensor_tensor(out=M2[:], in0=M1[:, 0:4 * C], in1=M1[:, 4 * C:],
                                op=mybir.AluOpType.max)
        M3 = tpool.tile([128, 2 * C], F16, tag="M3")
        nc.vector.tensor_tensor(out=M3[:], in0=M2[:, 0:2 * C], in1=M2[:, 2 * C:],
                                op=mybir.AluOpType.max)
        O = tpool.tile([128, C], F16, tag="O")
        nc.vector.tensor_tensor(out=O[:], in0=M3[:, 0:C], in1=M3[:, C:],
                                op=mybir.AluOpType.max)
        nc.sync.dma_start(out=out_o[bass.ts(tq, 128), :], in_=O[:])

    # ---- main loop over query tiles (software pipelined) ----
    # stage A(t): scores -> packed -> L1/L2 top-16 -> idxf
    # stage B(t): idx relayout (PE transposes) -> gtidx -> dma_gather
    # stage C(t): tail (A = G+u, stats, max over K, out DMA)
    # Emission A(t), B(t-1), C(t-2) keeps each in-order engine queue from
    # stalling tile t's independent work behind tile t-1's cross-engine chain.
    G_tiles = {}
    idx_tiles = {}

    def stageA(qt):
        qcols = bass.ts(qt, 128)
        pk = packed
        pk_hi = pk[:].bitcast(F16).rearrange("p (n two) -> p n two", two=2)

        # scores -> packed fp16-high-lane (+bias) -> L1 top-8 per chunk
        Ct = tk.tile([128, CAND], F32, tag="C")
        for c in range(NCH):
            ps_s = ps_score.tile([128, CH], F32, tag="score",
                                 name=f"pss{uniq}_{qt}_{c}")
            nc.tensor.matmul(ps_s[:, 0:512], lhsT=q_aug[:, qcols],
                             rhs=keys_aug[:, c * CH:c * CH + 512],
                             start=True, stop=True)
            nc.tensor.matmul(ps_s[:, 512:1024], lhsT=q_aug[:, qcols],
                             rhs=keys_aug[:, c * CH + 512:(c + 1) * CH],
                             start=True, stop=True)
            nc.scalar.activation(pk_hi[:, c * CH:(c + 1) * CH, 1:2], ps_s[:],
                                 mybir.ActivationFunctionType.Identity,
                                 bias=bias_all[:, qt:qt + 1], scale=1.0)
            nc.vector.max(out=Ct[:, bass.ts(c, 8)],
                          in_=pk[:, c * CH:(c + 1) * CH])

        # L2: top-16 packed words
        T12 = tk.tile([128, K], F32, tag="T12")
        Cmr = tk.tile([128, CAND], F32, tag="Cmr")
        nc.vector.max(out=T12[:, 0:8], in_=Ct[:])
        nc.vector.match_replace(out=Cmr[:], in_to_replace=T12[:, 0:8],
                                in_values=Ct[:], imm_value=-1e30)
        nc.vector.max(out=T12[:, 8:16], in_=Cmr[:])
        idxf = tk.tile([128, K], F32, tag="idxf")
        nc.vector.tensor_copy(
            out=idxf[:].unsqueeze(2),
            in_=T12[:].bitcast(U16).rearrange("p (n two) -> p n two",
                                              two=2)[:, :, 0:1])
        idx_tiles[qt] = idxf

    def stageB(qt):
        idxf = idx_tiles.pop(qt)
        # relayout idx [128q, 16k] -> wrap layout [16, 128]:
        # X[k, q] = idx[q, k] (one full transpose), then per-16-block
        # transposes ps_y[b, 16a+k] = X[k, 16a+b] (all base-partition 0).
        # The block transposes take a free-duplicated input so the output
        # lands twice (partitions 0:16 and 16:32) - the gather's tx Q7 core
        # reads the index rows from partitions 16:32.
        ps_x = ps_misc.tile([128, 512], F32, tag="misc", name=f"psx{uniq}_{qt}")
        nc.tensor.transpose(ps_x[0:16, 0:128], idxf[:], identity[:])
        Xsb = tk.tile([16, 128], F32, tag="Xsb")
        nc.scalar.copy(Xsb[:], ps_x[0:16, 0:128])
        ps_y = ps_misc.tile([128, 512], F32, tag="misc", name=f"psy{uniq}_{qt}")
        for a in range(8):
            nc.tensor.transpose(ps_y[0:16, 16 * a:16 * (a + 1)],
                                Xsb[:, 16 * a:16 * (a + 1)],
                                identity[0:16, 0:16])
        gt = gtidx
        # gtidx[b, 8k+a] = ps_y[b, 16a+k]; the gather's tx Q7 core reads the
        # index rows from partitions 16:32, so DMA-replicate them there.
        nc.scalar.activation(
            gt[0:16, :],
            ps_y[0:16, 0:128].rearrange("p (a k) -> p k a", a=8),
            mybir.ActivationFunctionType.Copy)
        nc.sync.dma_start(out=gt[16:32, :], in_=gt[0:16, :])

        # gather neighbor features G[q, k, :] = v^T[nn[q, k], :]
        # (4 ops of 512 idxs - the Q7 idx scratch caps num_idxs at 512)
        G = gpool.tile([128, K * C], F32, tag="G")
        Gv = G[:].rearrange("p (k o) -> p k o", k=K)
        for g in range(4):
            nc.gpsimd.dma_gather(
                out_ap=Gv[:, 4 * g:4 * (g + 1), :],
                in_ap=vt_dram,
                idxs_ap=gt[:, 32 * g:32 * (g + 1)],
                num_idxs=512,
                num_idxs_reg=512,
                elem_size=C,
            )
        G_tiles[qt] = G

    SL = cfg.get("stage_limit", 3)
    for qt in range(NQT):
        stageA(qt)
        if SL >= 2 and qt >= 1:
            stageB(qt - 1)
        if SL >= 3 and qt >= 2:
            tail(qt - 2)
    if SL >= 2:
        stageB(NQT - 1)
    if SL >= 3:
        tail(NQT - 2)
        tail(NQT - 1)

    # ---- epilogue: stats psum -> sbuf -> dram ----
    if SL >= 3:
        s_sb = big.tile([128, 512], F32, tag="s_sb")
        nc.vector.memset(s_sb[:], 0.0)
        for p in (0, 32, 64, 96):
            nc.scalar.copy(s_sb[p:p + 1, :], stats_ps[p:p + 1, :])
        nc.sync.dma_start(out=out_s, in_=s_sb[:])
    ctx.close()


def build_program(cfg, num_cores=8, reps=1):
    nc = bacc.Bacc("TRN2", target_bir_lowering=False, debug=False,
                   enable_asserts=False, num_devices=num_cores)
    C, NK, NQ = cfg["C"], cfg["NK"], cfg["NQ"]
    ins = {
        "f": nc.dram_tensor("f", [C, NK], F16, kind="ExternalInput").ap(),
        "fq": nc.dram_tensor("fq", [C, NQ], F16, kind="ExternalInput").ap(),
        "w2t": nc.dram_tensor("w2t", [C, C], F16, kind="ExternalInput").ap(),
        "wat": nc.dram_tensor("wat", [C, C], F16, kind="ExternalInput").ap(),
        "mb": nc.dram_tensor("mb", [128, 1], F16, kind="ExternalInput").ap(),
        "pkt": nc.dram_tensor("pkt", [128, NK], F32, kind="ExternalInput").ap(),
    }
    outs = {
        "out_o": nc.dram_tensor("out_o", [NQ, C], F16,
                                kind="ExternalOutput").ap(),
        "out_s": nc.dram_tensor("out_s", [128, 512], F32,
                                kind="ExternalOutput").ap(),
    }
    with tile.TileContext(nc) as tc:
        for r in range(reps):
            emit(tc, ins, outs, cfg, uniq=f"_r{r}")
    nc.compile()
    return nc


_PROGRAM_CACHE = {}


def get_program(num_cores=8):
    key = num_cores
    if key not in _PROGRAM_CACHE:
        _PROGRAM_CACHE[key] = build_program(default_cfg(), num_cores)
    return _PROGRAM_CACHE[key]


def make_in_maps(x, W, n_cores=8):
    """Build the per-core input dicts from the full inputs."""
    cfg = default_cfg()
    C, NQ = cfg["C"], cfg["NQ"]
    B, _, N, _ = x.shape
    per_batch = N // NQ
    f16 = np.ascontiguousarray(x[:, :, :, 0]).astype(np.float16)  # (B, C, N)
    W16 = W.astype(np.float16)
    W1, W2 = W16[:, :C], W16[:, C:]
    w2t = np.ascontiguousarray(W2.T)
    wat = np.ascontiguousarray((W1 - W2).T)
    pkt = np.broadcast_to(
        np.arange(N, dtype=np.uint32)[None, :], (128, N)).copy().view(np.float32)
    mbs = []
    for b in range(B):
        fb = f16[b].astype(np.float32)
        mbar = fb.mean(axis=1)                      # (C,)
        c0 = float(0.5 * (fb * fb).sum(axis=0).mean())
        mbv = np.zeros((128, 1), np.float16)
        mbv[0:C, 0] = (-mbar).astype(np.float16)
        # contracted against the -0.5 rows of q_aug: 64 * (-0.5) * (-c0/32) = c0
        mbv[C:128, 0] = np.float16(-c0 / 32.0)
        mbs.append(mbv)
    in_maps = []
    for c in range(n_cores):
        b, qb = c // per_batch, c % per_batch
        in_maps.append({
            "f": np.ascontiguousarray(f16[b]),
            "fq": np.ascontiguousarray(f16[b][:, qb * NQ:(qb + 1) * NQ]),
            "w2t": w2t,
            "wat": wat,
            "mb": mbs[b],
            "pkt": pkt,
        })
    return in_maps


def host_epilogue(m_full, s1, s2, gamma, beta, count):
    mean = s1 / count
    var = s2 / count - mean * mean
    a = gamma.astype(np.float64) / np.sqrt(var + BN_EPS)
    b = beta.astype(np.float64) - a * mean
    y = a[None, :, None] * m_full.astype(np.float64) + b[None, :, None]
    y = np.where(y >= 0, y, LRELU_SLOPE * y)
    return y.astype(np.float32)


def kernel(x, W, gamma, beta):
    """Full (unsharded) inputs -> full output. See module docstring."""
    from concourse import bass_utils

    x = np.asarray(x)
    W = np.asarray(W)
    gamma = np.asarray(gamma)
    beta = np.asarray(beta)

    B, C, N, _ = x.shape
    K = 16
    assert (B, C, N) == (2, 64, 8192), "kernel hardcoded for this problem size"

    cfg = default_cfg()
    NQ = cfg["NQ"]
    n_cores = 8
    per_batch = N // NQ

    in_maps = make_in_maps(x, W, n_cores)
    nc = get_program(n_cores)
    res = bass_utils.run_bass_kernel_spmd(nc, in_maps, list(range(n_cores)))
    results = res.results

    m_full = np.empty((B, C, N), np.float32)
    s1 = np.zeros(C, np.float64)
    s2 = np.zeros(C, np.float64)
    H = K * C // 2
    for c in range(n_cores):
        b, qb = c // per_batch, c % per_batch
        m_full[b, :, qb * NQ:(qb + 1) * NQ] = \
            results[c]["out_o"].astype(np.float32).T
        st = results[c]["out_s"].astype(np.float64)
        s1 += (st[0, :H].reshape(K // 2, C) + st[32, :H].reshape(K // 2, C)).sum(0)
        s2 += (st[64, :H].reshape(K // 2, C) + st[96, :H].reshape(K // 2, C)).sum(0)

    count = float(B) * N * K
    return host_epilogue(m_full, s1, s2, gamma, beta, count)


if __name__ == "__main__":
    sys.path.insert(0, os.path.dirname(os.path.abspath(__file__)))
    import reference

    inputs = {k: np.asarray(v) for k, v in reference.setup_inputs().items()}
    out = kernel(**inputs)
    import jax
    cpu = jax.devices("cpu")[0]
    with jax.default_device(cpu):
        exp = np.asarray(reference.reference(
            **{k: jax.device_put(v, cpu) for k, v in inputs.items()}))
    err = np.abs(out - exp)
    rel = np.linalg.norm(out - exp) / np.linalg.norm(exp)
    print("max abs err:", err.max(), "rel l2 err:", rel)



# revision 24
# speedup vs baseline: 1.3737x; 1.3737x over previous
"""Trainium2 Bass kernel for nn_DynConv2d (DGCNN EdgeConv layer).

Reference computation (B=2, C=64, N=8192, K=16, C_out=64):
  f = x[:,:,:,0]
  nn_idx = top-16 nearest neighbors by squared L2 over point features
  feat = concat([x_i, x_j - x_i])          # (B, 2C, N, K)
  y = W @ feat                             # 1x1 conv
  y = BatchNorm2d(y)  (training stats over (B,N,K))
  y = LeakyReLU(0.2)(y)
  out = max over K                         # (B, C_out, N)

Algebraic restructuring:
  * W @ [x_i; x_j - x_i] = u[:,i] + v[:,j] with u = (W1-W2)@f, v = W2@f.
  * BN+LeakyReLU is per-channel monotone, so max over K commutes; the kernel
    returns max_k(u+v_j) plus the BN batch stats (sum / sum-sq); the final
    affine + lrelu runs on host.
  * KNN score s = q.m - |m|^2/2 via a 128-contraction fp16 matmul: rows 0:64
    are features, rows 64:128 of keys hold -f^2/2 (q side holds ones), so no
    separate |m|^2 row materialization is needed.

Top-16 selection (per query row of 8192 fp32 PSUM scores):
  * Scalar engine evacuates PSUM -> SBUF converting to fp16 *into the high
    u16 lane* of a packed fp32 word whose low u16 lane is the key index
    (preloaded from a host template).  Numeric fp32 ordering of the packed
    word == lexicographic (fp16 score, index) ordering, so a single DVE
    Max top-8 per 1024-chunk yields values *and* indices in one pass - no
    MaxIndex, no scatter.
  * A per-query bias (-(q.mbar - c0 + 3.9|q|)) is added during evacuation to
    center the interesting (top-16) scores near zero, which shrinks the fp16
    rounding error where it matters.  Any per-query constant preserves the
    within-row order, so this never breaks correctness.
  * L2: Max + MatchReplace + Max over the 64 chunk-candidates -> top-16
    packed words; low lanes are the global key indices.
  * Indices are relayed out to the dma_gather wrap layout (16 partitions,
    idx[n%16, n//16] = nn-index of slot n = k*128+q) with 16 tiny 16x16 PE
    transposes, then one InstDMAGatherAnt fetches all 2048 neighbor rows of
    v^T per query tile.

Sharding: 8 cores; core c handles batch c//4, query block c%4 (2048 queries),
against all 8192 keys of its batch.
"""

import os
import sys

import numpy as np

sys.path.insert(0, "/opt/trn_rl_repo")

import concourse.bacc as bacc
import concourse.bass as bass
import concourse.mybir as mybir
import concourse.tile as tile
from concourse.masks import make_identity

F32 = mybir.dt.float32
F16 = mybir.dt.float16
U16 = mybir.dt.uint16
I16 = mybir.dt.int16

BN_EPS = 1e-5
LRELU_SLOPE = 0.2
BIAS_KAPPA = 3.9


def default_cfg():
    return dict(C=64, NK=8192, NQ=2048, K=16, CH=1024, B_DIST=1,
                TAIL_DIST=4)


def emit(tc, ins, outs, cfg, uniq=""):
    """Per-core program.

    ins:  f (64, NK) f16, fq (64, NQ) f16 (this core's query slice),
          w2t (C, C) f16, wat (C, C) f16,
          mb (128, 1) f16  [rows 0:64 = -mean_keys(f), rows 64:128 = c0/64],
          pkt (128, NK) f32 [u32 words: low u16 = column index, high = 0]
    outs: out_o (NQ, C) f16   max-over-K of u+v (pre-BN), query-major
          out_s (128, 512) f32  rows {0,32,64,96} = psum stats
    """
    nc = tc.nc
    C = cfg["C"]          # 64
    NK = cfg["NK"]        # 8192
    NQ = cfg["NQ"]        # 2048
    K = cfg["K"]          # 16
    CH = cfg["CH"]        # 1024  L1 top-8 chunk
    DB = cfg.get("B_DIST", 2)     # stageB(t-DB): idx relay + gather
    TD = cfg.get("TAIL_DIST", 4)  # tail(t-TD): A=G+u, stats, max, out
    # Every cross-engine dependency is >= 1 full iteration stale, so each
    # in-order engine queue runs back-to-back at its own pace and the
    # period is set by the busiest engine (DVE), not by the
    # Max->L2->transpose->gt->gather->add->stats relay chain.
    NQT = NQ // 128       # 16 query tiles
    NCH = NK // CH        # 8 chunks
    CAND = 8 * NCH        # 64 candidates
    H = K * C // 2        # 512 (stats half-width)

    f, fq, w2t, wat, mb, pkt = (ins["f"], ins["fq"], ins["w2t"], ins["wat"],
                                ins["mb"], ins["pkt"])
    out_o, out_s = outs["out_o"], outs["out_s"]

    from contextlib import ExitStack
    ctx = ExitStack()
    dram_pool = ctx.enter_context(tc.tile_pool(name="dram" + uniq, bufs=1,
                                               space="DRAM"))
    vt_dram = dram_pool.tile([NK, 2 * C], F16, tag="vt", name="vt_t" + uniq)[:]

    cpool = ctx.enter_context(tc.tile_pool(name="consts" + uniq, bufs=1))
    big = ctx.enter_context(tc.tile_pool(name="big" + uniq, bufs=1))
    tk = ctx.enter_context(tc.tile_pool(name="topk" + uniq, bufs=3))
    vpool = ctx.enter_context(tc.tile_pool(name="vstage" + uniq, bufs=8))
    gpool = ctx.enter_context(tc.tile_pool(name="gather" + uniq, bufs=4))
    tpool = ctx.enter_context(tc.tile_pool(name="tail" + uniq, bufs=4))
    ps_score = ctx.enter_context(tc.tile_pool(name="ps_score" + uniq, bufs=2,
                                              space="PSUM"))
    ps_stat = ctx.enter_context(tc.tile_pool(name="ps_stat" + uniq, bufs=1,
                                             space="PSUM"))
    ps_misc = ctx.enter_context(tc.tile_pool(name="ps_misc" + uniq, bufs=3,
                                             space="PSUM"))

    # ---- constants / inputs ----
    identity = cpool.tile([128, 128], F32, tag="ident")
    make_identity(nc, identity[:])
    ones64 = cpool.tile([64, 1], F16, tag="ones64")
    nc.vector.memset(ones64[:], 1.0)
    ones128 = cpool.tile([128, 1], F16, tag="ones128")
    nc.vector.memset(ones128[:], 1.0)
    w2t_sb = cpool.tile([128, C], F16, tag="w2t")
    wat_sb = cpool.tile([C, C], F16, tag="wat")
    mb_sb = cpool.tile([128, 1], F16, tag="mb")

    # keys_aug: rows 0:64 = f, rows 64:128 = -f^2/2 (built in place from a
    # second copy of f so every engine op stays partition-aligned).
    # DMAs are spread across engine queues so the prologue isn't serialized
    # on SP.  Prologue engine placement keeps the steady-state bottleneck
    # (DVE) light: f^2 squaring runs on DVE (idle in prologue, 2-byte 2x
    # path), v^T/u^T PSUM evacuation on Act.
    keys_aug = big.tile([128, NK], F16, tag="keys_aug")
    q_aug = big.tile([128, NQ], F16, tag="q_aug")
    NQR = NK // 4
    # keys2 r0 is the head of the prologue critical chain (vt matmuls ->
    # f^2 -> first score matmul); it leads the Act DMA queue
    for r in range(4):
        nc.scalar.dma_start(out=keys_aug[64:128, bass.ts(r, NQR)],
                            in_=f[:, bass.ts(r, NQR)])
    nc.sync.dma_start(out=q_aug[0:64, :], in_=fq)
    # w2t lives at base partition 64 to pair with keys_aug[64:128] in matmuls
    nc.sync.dma_start(out=w2t_sb[64:128, :], in_=w2t)
    nc.sync.dma_start(out=wat_sb[:], in_=wat)
    nc.sync.dma_start(out=mb_sb[:], in_=mb)
    # rows 64:128 hold -0.5 so the score matmul contracts -0.5 * f^2 rows
    nc.gpsimd.memset(q_aug[64:128, :], -0.5)

    # fq^2 for the per-query |q| bias estimate (DVE, 2-byte fast path)
    q2 = big.tile([64, NQ], F16, tag="q2")
    nc.vector.tensor_tensor(out=q2[:], in0=q_aug[0:64, :], in1=q_aug[0:64, :],
                            op=mybir.AluOpType.mult)

    # Single packed score buffer (low u16 lanes = key index template).  The
    # evac(t+1, c) -> Max(t, c) WAR is 8 chunk-slots stale, so one buffer
    # pipelines with no stalls.  keys1 quarters and template chunks
    # interleave on SP so chunk c's template lands just before Max(0, c).
    packed = big.tile([128, NK], F32, tag="packed")
    for r in range(4):
        nc.sync.dma_start(out=keys_aug[0:64, bass.ts(r, NQR)],
                          in_=f[:, bass.ts(r, NQR)])
    # gather index tiles (rows 16:128 must stay 0 for the executor's
    # bounds check; only rows 0:32 are consumed).  Two buffers so tile t's
    # index relay never waits for tile t-1's gathers to finish reading.
    gtidx2 = []
    for gi in range(2):
        gx = big.tile([128, 128], I16, tag=f"gtidx{gi}")
        nc.gpsimd.memset(gx[:], 0)
        gtidx2.append(gx)
    # index template rides the otherwise-idle Pool DMA queue; chunk c only
    # needs to land before Max(0, c), so only c0/c1 contend with the
    # critical keys transfers - the rest are issued after the vt loop
    for c in range(2):
        nc.gpsimd.dma_start(out=packed[:, c * CH:(c + 1) * CH],
                            in_=pkt[:, c * CH:(c + 1) * CH])

    # ---- v^T matmuls (read raw f in rows 64:128) + quartered -f^2/2 ----
    # Emission order per quarter: vt matmuls reading that quarter first, then
    # the in-place squaring of the same columns (readers before writer).
    vt_view = vt_dram.rearrange("(t c p) ch -> t p c ch", t=8, c=8)
    for t in range(8):
        ps_vt = ps_misc.tile([128, 512], F32, tag="misc")
        for c in range(8):
            nc.tensor.matmul(ps_vt[:, c * C:(c + 1) * C],
                             lhsT=keys_aug[64:128, bass.ts(8 * t + c, 128)],
                             rhs=w2t_sb[64:128, :], start=True, stop=True)
        vt_sb = vpool.tile([128, 512], F16, tag="vt_sb")
        nc.vector.tensor_copy(out=vt_sb[:], in_=ps_vt[:])
        nc.sync.dma_start(out=vt_view[t][:, :, 0:C],
                          in_=vt_sb[:].rearrange("p (c ch) -> p c ch", c=8))
        nc.sync.dma_start(out=vt_view[t][:, :, C:2 * C],
                          in_=vt_sb[:].rearrange("p (c ch) -> p c ch", c=8))
        if t % 2 == 1:
            r = t // 2
            nc.vector.tensor_tensor(
                out=keys_aug[64:128, bass.ts(r, NQR)],
                in0=keys_aug[64:128, bass.ts(r, NQR)],
                in1=keys_aug[64:128, bass.ts(r, NQR)],
                op=mybir.AluOpType.mult)
            for c in (2 * r + 2, 2 * r + 3):
                if c < NCH:
                    nc.gpsimd.dma_start(out=packed[:, c * CH:(c + 1) * CH],
                                        in_=pkt[:, c * CH:(c + 1) * CH])

    # per-query bias for all tiles: bias_all[:, t] = -(q.mbar - c0 + kappa*|q|)
    ps_ball = ps_misc.tile([128, 512], F32, tag="misc", name="psball" + uniq)
    for qt in range(NQT):
        nc.tensor.matmul(ps_ball[:, qt:qt + 1], lhsT=q_aug[:, bass.ts(qt, 128)],
                         rhs=mb_sb[:], start=True, stop=True)
        nc.tensor.matmul(ps_ball[:, NQT + qt:NQT + qt + 1],
                         lhsT=q2[:, bass.ts(qt, 128)], rhs=ones64[:],
                         start=True, stop=True)
    qn_all = big.tile([128, NQT], F32, tag="qn_all")
    nc.scalar.activation(qn_all[:], ps_ball[:, NQT:2 * NQT],
                         mybir.ActivationFunctionType.Sqrt, scale=1.0)
    bias_all = big.tile([128, NQT], F32, tag="bias_all")
    nc.vector.scalar_tensor_tensor(
        out=bias_all[:], in0=qn_all[:], scalar=-BIAS_KAPPA,
        in1=ps_ball[:, 0:NQT],
        op0=mybir.AluOpType.mult, op1=mybir.AluOpType.add)

    ut_sb = big.tile([128, NQT * C], F16, tag="ut")
    for h in range(2):
        ps_ut = ps_misc.tile([128, 512], F32, tag="misc")
        for j in range(8):
            nc.tensor.matmul(ps_ut[:, j * C:(j + 1) * C],
                             lhsT=q_aug[0:64, bass.ts(8 * h + j, 128)],
                             rhs=wat_sb[:], start=True, stop=True)
        nc.scalar.copy(ut_sb[:, h * 512:(h + 1) * 512], ps_ut[:])

    stats_ps = ps_stat.tile([128, 512], F32, tag="stat")

    # ---- per-tile tail, split so every engine's inputs are a full
    # iteration old when its queue reaches them ----
    # tailP(t): Pool A = G + u, A2 = A^2 (emitted FIRST in the iteration so
    #           they precede the gathers in Pool's in-order queue)
    # tailC(t): PE stats matmuls + DVE max-over-K fold + out DMA (reads
    #           A/A2 produced one iteration earlier)
    def tailP(tq):
        G = G_tiles.pop(tq)
        ut_qt = ut_sb[:, bass.ts(tq, C)]
        A = tpool.tile([128, K * C], F16, tag="A")
        nc.vector.tensor_tensor(
            out=A[:].rearrange("p (k o) -> p k o", k=K),
            in0=G[:].rearrange("p (k o) -> p k o", k=K)[:, :, 0:C],
            in1=ut_qt.unsqueeze(1).broadcast_to([128, K, C]),
            op=mybir.AluOpType.add)
        A_tiles[tq] = A

    def tailQ(tq):
        A = A_tiles[tq]
        A2 = tpool.tile([128, K * C], F16, tag="A2")
        # last tiles' squares land in the drain, where DVE is idle but Pool
        # still has gathers queued - run them on DVE there
        eng = nc.vector if tq >= NQT - 4 else nc.gpsimd
        eng.tensor_tensor(out=A2[:], in0=A[:], in1=A[:],
                          op=mybir.AluOpType.mult)
        A2_tiles[tq] = A2

    def tailC(tq):
        A = A_tiles.pop(tq)
        A2 = A2_tiles.pop(tq)
        st, sp = (tq == 0), (tq == NQT - 1)
        nc.tensor.matmul(stats_ps[0:1, 0:H], lhsT=ones128[:], rhs=A[:, 0:H],
                         start=st, stop=sp, tile_position=(0, 0))
        nc.tensor.matmul(stats_ps[32:33, 0:H], lhsT=ones128[:], rhs=A[:, H:],
                         start=st, stop=sp, tile_position=(0, 32))
        nc.tensor.matmul(stats_ps[64:65, 0:H], lhsT=ones128[:], rhs=A2[:, 0:H],
                         start=st, stop=sp, tile_position=(0, 64))
        nc.tensor.matmul(stats_ps[96:97, 0:H], lhsT=ones128[:], rhs=A2[:, H:],
                         start=st, stop=sp, tile_position=(0, 96))
        # max over K: fold tree (8,4,2,1) on DVE fp16 (2x path; Pool has no
        # ISA max, and contiguous halves keep the 2-byte fast mode)
        M1 = tpool.tile([128, 8 * C], F16, tag="M1")
        nc.vector.tensor_tensor(out=M1[:], in0=A[:, 0:8 * C], in1=A[:, 8 * C:],
                                op=mybir.AluOpType.max)
        M2 = tpool.tile([128, 4 * C], F16, tag="M2")
        nc.vector.tensor_tensor(out=M2[:], in0=M1[:, 0:4 * C], in1=M1[:, 4 * C:],
                                op=mybir.AluOpType.max)
        M3 = tpool.tile([128, 2 * C], F16, tag="M3")
        nc.vector.tensor_tensor(out=M3[:], in0=M2[:, 0:2 * C], in1=M2[:, 2 * C:],
                                op=mybir.AluOpType.max)
        O = tpool.tile([128, C], F16, tag="O")
        nc.vector.tensor_tensor(out=O[:], in0=M3[:, 0:C], in1=M3[:, C:],
                                op=mybir.AluOpType.max)
        # out DMA rides SP, whose queue has nothing else in steady state,
        # so its wait on O can't block any other work.
        nc.sync.dma_start(out=out_o[bass.ts(tq, 128), :], in_=O[:])

    # ---- main loop over query tiles (software pipelined) ----
    # stage A(t): scores -> packed -> L1/L2 top-16 -> idxf
    # stage B(t): idx relayout (PE transposes) -> gtidx -> dma_gather
    G_tiles = {}
    idx_tiles = {}
    Xsb_tiles = {}
    A_tiles = {}
    A2_tiles = {}

    def stageA(qt):
        qcols = bass.ts(qt, 128)
        pk = packed
        pk_hi = pk[:].bitcast(F16).rearrange("p (n two) -> p n two", two=2)

        # scores -> packed fp16-high-lane (+bias) -> L1 top-8 per chunk
        Ct = tk.tile([128, CAND], F32, tag="C")
        for c in range(NCH):
            ps_s = ps_score.tile([128, CH], F32, tag="score",
                                 name=f"pss{uniq}_{qt}_{c}")
            nc.tensor.matmul(ps_s[:, 0:512], lhsT=q_aug[:, qcols],
                             rhs=keys_aug[:, c * CH:c * CH + 512],
                             start=True, stop=True)
            nc.tensor.matmul(ps_s[:, 512:1024], lhsT=q_aug[:, qcols],
                             rhs=keys_aug[:, c * CH + 512:(c + 1) * CH],
                             start=True, stop=True)
            nc.scalar.activation(pk_hi[:, c * CH:(c + 1) * CH, 1:2], ps_s[:],
                                 mybir.ActivationFunctionType.Identity,
                                 bias=bias_all[:, qt:qt + 1], scale=1.0)
            nc.vector.max(out=Ct[:, bass.ts(c, 8)],
                          in_=pk[:, c * CH:(c + 1) * CH])

        # L2: top-16 packed words
        T12 = tk.tile([128, K], F32, tag="T12")
        Cmr = tk.tile([128, CAND], F32, tag="Cmr")
        nc.vector.max(out=T12[:, 0:8], in_=Ct[:])
        nc.vector.match_replace(out=Cmr[:], in_to_replace=T12[:, 0:8],
                                in_values=Ct[:], imm_value=-1e30)
        nc.vector.max(out=T12[:, 8:16], in_=Cmr[:])
        idxf = tk.tile([128, K], F32, tag="idxf")
        nc.vector.tensor_copy(
            out=idxf[:].unsqueeze(2),
            in_=T12[:].bitcast(U16).rearrange("p (n two) -> p n two",
                                              two=2)[:, :, 0:1])
        idx_tiles[qt] = idxf

    def stageB1(qt):
        idxf = idx_tiles.pop(qt)
        # relayout idx [128q, 16k] -> wrap layout [16, 128]:
        # X[k, q] = idx[q, k] (one full transpose).  PE transpose runs after
        # this iteration's score matmuls; the Act copy sits after this
        # iteration's evacs, by which time ps_x is long done.
        ps_x = ps_misc.tile([128, 512], F32, tag="misc", name=f"psx{uniq}_{qt}")
        nc.tensor.transpose(ps_x[0:16, 0:128], idxf[:], identity[:])
        Xsb = tk.tile([16, 128], F32, tag="Xsb")
        nc.scalar.copy(Xsb[:], ps_x[0:16, 0:128])
        Xsb_tiles[qt] = Xsb

    def stageB2(qt):
        Xsb = Xsb_tiles.pop(qt)
        # per-16-block transposes ps_y[b, 16a+k] = X[k, 16a+b] (all
        # base-partition 0).  The block transposes take a free-duplicated
        # input so the output lands twice (partitions 0:16 and 16:32) - the
        # gather's tx Q7 core reads the index rows from partitions 16:32.
        ps_y = ps_misc.tile([128, 512], F32, tag="misc", name=f"psy{uniq}_{qt}")
        for a in range(8):
            nc.tensor.transpose(ps_y[0:16, 16 * a:16 * (a + 1)],
                                Xsb[:, 16 * a:16 * (a + 1)],
                                identity[0:16, 0:16])
        gt = gtidx2[qt % 2]
        # gtidx[b, 8k+a] = ps_y[b, 16a+k]; the gather's tx Q7 core reads the
        # index rows from partitions 16:32, so DMA-replicate them there.
        nc.scalar.activation(
            gt[0:16, :],
            ps_y[0:16, 0:128].rearrange("p (a k) -> p k a", a=8),
            mybir.ActivationFunctionType.Copy)
        # replicate rides the Act queue right after the copy that feeds it
        nc.scalar.dma_start(out=gt[16:32, :], in_=gt[0:16, :])

        # gather neighbor features G[q, k, 0:C] = v^T[nn[q, k], :]
        # (4 ops of 512 idxs - the Q7 idx scratch caps num_idxs at 512;
        #  f16 rows padded to 256 bytes to satisfy the descriptor-size rule)
        G = gpool.tile([128, K * 2 * C], F16, tag="G")
        Gv = G[:].rearrange("p (k o) -> p k o", k=K)
        for g in range(4):
            nc.gpsimd.dma_gather(
                out_ap=Gv[:, 4 * g:4 * (g + 1), :],
                in_ap=vt_dram,
                idxs_ap=gt[:, 32 * g:32 * (g + 1)],
                num_idxs=512,
                num_idxs_reg=512,
                elem_size=2 * C,
            )
        G_tiles[qt] = G

    # Virtual-iteration schedule.  Stage offsets (it = virtual iteration):
    #   stageA(it)     scores/evac/L1+L2 top-k   PE/Act/DVE
    #   stageB1(it-2)  idx transpose + Xsb       PE tail / Act tail
    #   stageB2(it-3)  blocks + gt + gathers     PE / Act tail / Pool tail
    #   tailP(it-4)    A = G+u, A2               Pool head
    #   tailC(it-6)    stats + fold + out        PE head / DVE head / SP
    # All cross-engine inputs are >= 1 iteration old except the intended
    # mm -> evac -> Max chunk pipeline and the same-iteration PE -> Act
    # relay (PE runs those ~4us before Act reaches them).
    SL = cfg.get("stage_limit", 3)
    TC = TD + 2
    for it in range(NQT + TC):
        if SL >= 3 and 0 <= it - TC:
            tailC(it - TC)
        if SL >= 3 and 0 <= it - TD - 1 < NQT:
            tailQ(it - TD - 1)
        if SL >= 3 and 0 <= it - TD < NQT:
            tailP(it - TD)
        if it < NQT:
            stageA(it)
        if SL >= 2 and 0 <= it - DB < NQT:
            stageB1(it - DB)
        if SL >= 2 and 0 <= it - DB - 1 < NQT:
            stageB2(it - DB - 1)

    # ---- epilogue: stats psum -> sbuf -> dram ----
    if SL >= 3:
        s_sb = big.tile([128, 512], F32, tag="s_sb")
        nc.vector.memset(s_sb[:], 0.0)
        for p in (0, 32, 64, 96):
            nc.scalar.copy(s_sb[p:p + 1, :], stats_ps[p:p + 1, :])
        nc.sync.dma_start(out=out_s, in_=s_sb[:])
    ctx.close()


def build_program(cfg, num_cores=8, reps=1):
    nc = bacc.Bacc("TRN2", target_bir_lowering=False, debug=False,
                   enable_asserts=False, num_devices=num_cores)
    C, NK, NQ = cfg["C"], cfg["NK"], cfg["NQ"]
    ins = {
        "f": nc.dram_tensor("f", [C, NK], F16, kind="ExternalInput").ap(),
        "fq": nc.dram_tensor("fq", [C, NQ], F16, kind="ExternalInput").ap(),
        "w2t": nc.dram_tensor("w2t", [C, C], F16, kind="ExternalInput").ap(),
        "wat": nc.dram_tensor("wat", [C, C], F16, kind="ExternalInput").ap(),
        "mb": nc.dram_tensor("mb", [128, 1], F16, kind="ExternalInput").ap(),
        "pkt": nc.dram_tensor("pkt", [128, NK], F32, kind="ExternalInput").ap(),
    }
    outs = {
        "out_o": nc.dram_tensor("out_o", [NQ, C], F16,
                                kind="ExternalOutput").ap(),
        "out_s": nc.dram_tensor("out_s", [128, 512], F32,
                                kind="ExternalOutput").ap(),
    }
    with tile.TileContext(nc) as tc:
        for r in range(reps):
            emit(tc, ins, outs, cfg, uniq=f"_r{r}")
    nc.compile()
    return nc


_PROGRAM_CACHE = {}


def get_program(num_cores=8):
    key = num_cores
    if key not in _PROGRAM_CACHE:
        _PROGRAM_CACHE[key] = build_program(default_cfg(), num_cores)
    return _PROGRAM_CACHE[key]


def make_in_maps(x, W, n_cores=8):
    """Build the per-core input dicts from the full inputs."""
    cfg = default_cfg()
    C, NQ = cfg["C"], cfg["NQ"]
    B, _, N, _ = x.shape
    per_batch = N // NQ
    f16 = np.ascontiguousarray(x[:, :, :, 0]).astype(np.float16)  # (B, C, N)
    W16 = W.astype(np.float16)
    W1, W2 = W16[:, :C], W16[:, C:]
    w2t = np.ascontiguousarray(W2.T)
    wat = np.ascontiguousarray((W1 - W2).T)
    pkt = np.broadcast_to(
        np.arange(N, dtype=np.uint32)[None, :], (128, N)).copy().view(np.float32)
    mbs = []
    for b in range(B):
        fb = f16[b].astype(np.float32)
        mbar = fb.mean(axis=1)                      # (C,)
        c0 = float(0.5 * (fb * fb).sum(axis=0).mean())
        mbv = np.zeros((128, 1), np.float16)
        mbv[0:C, 0] = (-mbar).astype(np.float16)
        # contracted against the -0.5 rows of q_aug: 64 * (-0.5) * (-c0/32) = c0
        mbv[C:128, 0] = np.float16(-c0 / 32.0)
        mbs.append(mbv)
    in_maps = []
    for c in range(n_cores):
        b, qb = c // per_batch, c % per_batch
        in_maps.append({
            "f": np.ascontiguousarray(f16[b]),
            "fq": np.ascontiguousarray(f16[b][:, qb * NQ:(qb + 1) * NQ]),
            "w2t": w2t,
            "wat": wat,
            "mb": mbs[b],
            "pkt": pkt,
        })
    return in_maps


def host_epilogue(m_full, s1, s2, gamma, beta, count):
    mean = s1 / count
    var = s2 / count - mean * mean
    a = gamma.astype(np.float64) / np.sqrt(var + BN_EPS)
    b = beta.astype(np.float64) - a * mean
    y = a[None, :, None] * m_full.astype(np.float64) + b[None, :, None]
    y = np.where(y >= 0, y, LRELU_SLOPE * y)
    return y.astype(np.float32)


def kernel(x, W, gamma, beta):
    """Full (unsharded) inputs -> full output. See module docstring."""
    from concourse import bass_utils

    x = np.asarray(x)
    W = np.asarray(W)
    gamma = np.asarray(gamma)
    beta = np.asarray(beta)

    B, C, N, _ = x.shape
    K = 16
    assert (B, C, N) == (2, 64, 8192), "kernel hardcoded for this problem size"

    cfg = default_cfg()
    NQ = cfg["NQ"]
    n_cores = 8
    per_batch = N // NQ

    in_maps = make_in_maps(x, W, n_cores)
    nc = get_program(n_cores)
    res = bass_utils.run_bass_kernel_spmd(nc, in_maps, list(range(n_cores)))
    results = res.results

    m_full = np.empty((B, C, N), np.float32)
    s1 = np.zeros(C, np.float64)
    s2 = np.zeros(C, np.float64)
    H = K * C // 2
    for c in range(n_cores):
        b, qb = c // per_batch, c % per_batch
        m_full[b, :, qb * NQ:(qb + 1) * NQ] = \
            results[c]["out_o"].astype(np.float32).T
        st = results[c]["out_s"].astype(np.float64)
        s1 += (st[0, :H].reshape(K // 2, C) + st[32, :H].reshape(K // 2, C)).sum(0)
        s2 += (st[64, :H].reshape(K // 2, C) + st[96, :H].reshape(K // 2, C)).sum(0)

    count = float(B) * N * K
    return host_epilogue(m_full, s1, s2, gamma, beta, count)


if __name__ == "__main__":
    sys.path.insert(0, os.path.dirname(os.path.abspath(__file__)))
    import reference

    inputs = {k: np.asarray(v) for k, v in reference.setup_inputs().items()}
    out = kernel(**inputs)
    import jax
    cpu = jax.devices("cpu")[0]
    with jax.default_device(cpu):
        exp = np.asarray(reference.reference(
            **{k: jax.device_put(v, cpu) for k, v in inputs.items()}))
    err = np.abs(out - exp)
    rel = np.linalg.norm(out - exp) / np.linalg.norm(exp)
    print("max abs err:", err.max(), "rel l2 err:", rel)



# revision 26
# speedup vs baseline: 1.4702x; 1.0702x over previous
"""Trainium2 Bass kernel for nn_DynConv2d (DGCNN EdgeConv layer).

Reference computation (B=2, C=64, N=8192, K=16, C_out=64):
  f = x[:,:,:,0]
  nn_idx = top-16 nearest neighbors by squared L2 over point features
  feat = concat([x_i, x_j - x_i])          # (B, 2C, N, K)
  y = W @ feat                             # 1x1 conv
  y = BatchNorm2d(y)  (training stats over (B,N,K))
  y = LeakyReLU(0.2)(y)
  out = max over K                         # (B, C_out, N)

Algebraic restructuring:
  * W @ [x_i; x_j - x_i] = u[:,i] + v[:,j] with u = (W1-W2)@f, v = W2@f.
  * BN+LeakyReLU is per-channel monotone, so max over K commutes; the kernel
    returns max_k(u+v_j) plus the BN batch stats (sum / sum-sq); the final
    affine + lrelu runs on host.
  * KNN score s = q.m - |m|^2/2 via a 128-contraction fp16 matmul: rows 0:64
    are features, rows 64:128 of keys hold -f^2/2 (q side holds ones), so no
    separate |m|^2 row materialization is needed.

Top-16 selection (per query row of 8192 fp32 PSUM scores):
  * Scalar engine evacuates PSUM -> SBUF converting to fp16 *into the high
    u16 lane* of a packed fp32 word whose low u16 lane is the key index
    (preloaded from a host template).  Numeric fp32 ordering of the packed
    word == lexicographic (fp16 score, index) ordering, so a single DVE
    Max top-8 per 1024-chunk yields values *and* indices in one pass - no
    MaxIndex, no scatter.
  * A per-query bias (-(q.mbar - c0 + 3.9|q|)) is added during evacuation to
    center the interesting (top-16) scores near zero, which shrinks the fp16
    rounding error where it matters.  Any per-query constant preserves the
    within-row order, so this never breaks correctness.
  * L2: Max + MatchReplace + Max over the 64 chunk-candidates -> top-16
    packed words; low lanes are the global key indices.
  * Indices are relayed out to the dma_gather wrap layout (16 partitions,
    idx[n%16, n//16] = nn-index of slot n = k*128+q) with 16 tiny 16x16 PE
    transposes, then one InstDMAGatherAnt fetches all 2048 neighbor rows of
    v^T per query tile.

Sharding: 8 cores; core c handles batch c//4, query block c%4 (2048 queries),
against all 8192 keys of its batch.
"""

import os
import sys

import numpy as np

sys.path.insert(0, "/opt/trn_rl_repo")

import concourse.bacc as bacc
import concourse.bass as bass
import concourse.mybir as mybir
import concourse.tile as tile
from concourse.masks import make_identity

F32 = mybir.dt.float32
F16 = mybir.dt.float16
U16 = mybir.dt.uint16
I16 = mybir.dt.int16

BN_EPS = 1e-5
LRELU_SLOPE = 0.2
BIAS_KAPPA = 3.9


def default_cfg():
    return dict(C=64, NK=8192, NQ=2048, K=16, CH=1024, B_DIST=1,
                TAIL_DIST=4)


def emit(tc, ins, outs, cfg, uniq=""):
    """Per-core program.

    ins:  f (64, NK) f16, fq (64, NQ) f16 (this core's query slice),
          w2t (C, C) f16, wat (C, C) f16,
          mb (128, 1) f16  [rows 0:64 = -mean_keys(f), rows 64:128 = c0/64],
          pkt (128, NK) f32 [u32 words: low u16 = column index, high = 0]
    outs: out_o (NQ, C) f16   max-over-K of u+v (pre-BN), query-major
          out_s (128, 512) f32  rows {0,32,64,96} = psum stats
    """
    nc = tc.nc
    C = cfg["C"]          # 64
    NK = cfg["NK"]        # 8192
    NQ = cfg["NQ"]        # 2048
    K = cfg["K"]          # 16
    CH = cfg["CH"]        # 1024  L1 top-8 chunk
    DB = cfg.get("B_DIST", 2)     # stageB(t-DB): idx relay + gather
    TD = cfg.get("TAIL_DIST", 4)  # tail(t-TD): A=G+u, stats, max, out
    # Every cross-engine dependency is >= 1 full iteration stale, so each
    # in-order engine queue runs back-to-back at its own pace and the
    # period is set by the busiest engine (DVE), not by the
    # Max->L2->transpose->gt->gather->add->stats relay chain.
    NQT = NQ // 128       # 16 query tiles
    NCH = NK // CH        # 8 chunks
    CAND = 8 * NCH        # 64 candidates
    H = K * C // 2        # 512 (stats half-width)

    f, fq, w2t, wat, mb, pkt = (ins["f"], ins["fq"], ins["w2t"], ins["wat"],
                                ins["mb"], ins["pkt"])
    out_o, out_s = outs["out_o"], outs["out_s"]

    from contextlib import ExitStack
    ctx = ExitStack()
    dram_pool = ctx.enter_context(tc.tile_pool(name="dram" + uniq, bufs=1,
                                               space="DRAM"))
    vt_dram = dram_pool.tile([NK, 2 * C], F16, tag="vt", name="vt_t" + uniq)[:]

    cpool = ctx.enter_context(tc.tile_pool(name="consts" + uniq, bufs=1))
    big = ctx.enter_context(tc.tile_pool(name="big" + uniq, bufs=1))
    tk = ctx.enter_context(tc.tile_pool(name="topk" + uniq, bufs=3))
    vpool = ctx.enter_context(tc.tile_pool(name="vstage" + uniq, bufs=8))
    gpool = ctx.enter_context(tc.tile_pool(name="gather" + uniq, bufs=4))
    tpool = ctx.enter_context(tc.tile_pool(name="tail" + uniq, bufs=4))
    ps_score = ctx.enter_context(tc.tile_pool(name="ps_score" + uniq, bufs=2,
                                              space="PSUM"))
    ps_stat = ctx.enter_context(tc.tile_pool(name="ps_stat" + uniq, bufs=1,
                                             space="PSUM"))
    ps_misc = ctx.enter_context(tc.tile_pool(name="ps_misc" + uniq, bufs=3,
                                             space="PSUM"))

    # ---- constants / inputs ----
    identity = cpool.tile([128, 128], F32, tag="ident")
    make_identity(nc, identity[:])
    ones64 = cpool.tile([64, 1], F16, tag="ones64")
    nc.vector.memset(ones64[:], 1.0)
    ones128 = cpool.tile([128, 1], F16, tag="ones128")
    nc.vector.memset(ones128[:], 1.0)
    w2t_sb = cpool.tile([128, C], F16, tag="w2t")
    wat_sb = cpool.tile([C, C], F16, tag="wat")
    mb_sb = cpool.tile([128, 1], F16, tag="mb")

    # keys_aug: rows 0:64 = f, rows 64:128 = -f^2/2 (built in place from a
    # second copy of f so every engine op stays partition-aligned).
    # DMAs are spread across engine queues so the prologue isn't serialized
    # on SP.  Prologue engine placement keeps the steady-state bottleneck
    # (DVE) light: f^2 squaring runs on DVE (idle in prologue, 2-byte 2x
    # path), v^T/u^T PSUM evacuation on Act.
    keys_aug = big.tile([128, NK], F16, tag="keys_aug")
    q_aug = big.tile([128, NQ], F16, tag="q_aug")
    NQR = NK // 4
    # keys2 r0 is the head of the prologue critical chain (vt matmuls ->
    # f^2 -> first score matmul); it leads the Act DMA queue
    for r in range(4):
        nc.scalar.dma_start(out=keys_aug[64:128, bass.ts(r, NQR)],
                            in_=f[:, bass.ts(r, NQR)])
    nc.sync.dma_start(out=q_aug[0:64, :], in_=fq)
    # w2t lives at base partition 64 to pair with keys_aug[64:128] in matmuls
    nc.sync.dma_start(out=w2t_sb[64:128, :], in_=w2t)
    nc.sync.dma_start(out=wat_sb[:], in_=wat)
    nc.sync.dma_start(out=mb_sb[:], in_=mb)
    # rows 64:128 hold -0.5 so the score matmul contracts -0.5 * f^2 rows
    nc.gpsimd.memset(q_aug[64:128, :], -0.5)

    # fq^2 for the per-query |q| bias estimate (DVE, 2-byte fast path)
    q2 = big.tile([64, NQ], F16, tag="q2")
    nc.vector.tensor_tensor(out=q2[:], in0=q_aug[0:64, :], in1=q_aug[0:64, :],
                            op=mybir.AluOpType.mult)

    # Single packed score buffer (low u16 lanes = key index template).  The
    # evac(t+1, c) -> Max(t, c) WAR is 8 chunk-slots stale, so one buffer
    # pipelines with no stalls.  keys1 quarters and template chunks
    # interleave on SP so chunk c's template lands just before Max(0, c).
    packed2 = [big.tile([128, NK], F32, tag=f"packed{i}",
                        name=f"packed{i}" + uniq) for i in range(2)]
    for r in range(4):
        nc.sync.dma_start(out=keys_aug[0:64, bass.ts(r, NQR)],
                          in_=f[:, bass.ts(r, NQR)])
    # gather index tiles (rows 16:128 must stay 0 for the executor's
    # bounds check; only rows 0:32 are consumed).  Two buffers so tile t's
    # index relay never waits for tile t-1's gathers to finish reading.
    gtidx2 = []
    for gi in range(2):
        gx = big.tile([128, 128], I16, tag=f"gtidx{gi}")
        nc.gpsimd.memset(gx[:], 0)
        gtidx2.append(gx)
    # index template rides the otherwise-idle Pool DMA queue; chunk c only
    # needs to land before Max(0, c), so only c0/c1 contend with the
    # critical keys transfers - the rest are issued after the vt loop
    for c in range(2):
        for pb in packed2:
            nc.gpsimd.dma_start(out=pb[:, c * CH:(c + 1) * CH],
                                in_=pkt[:, c * CH:(c + 1) * CH])

    # ---- v^T matmuls (read raw f in rows 64:128) + quartered -f^2/2 ----
    # Emission order per quarter: vt matmuls reading that quarter first, then
    # the in-place squaring of the same columns (readers before writer).
    vt_view = vt_dram.rearrange("(t c p) ch -> t p c ch", t=8, c=8)
    for t in range(8):
        ps_vt = ps_misc.tile([128, 512], F32, tag="misc")
        for c in range(8):
            nc.tensor.matmul(ps_vt[:, c * C:(c + 1) * C],
                             lhsT=keys_aug[64:128, bass.ts(8 * t + c, 128)],
                             rhs=w2t_sb[64:128, :], start=True, stop=True)
        vt_sb = vpool.tile([128, 512], F16, tag="vt_sb")
        nc.vector.tensor_copy(out=vt_sb[:], in_=ps_vt[:])
        nc.sync.dma_start(out=vt_view[t][:, :, 0:C],
                          in_=vt_sb[:].rearrange("p (c ch) -> p c ch", c=8))
        nc.sync.dma_start(out=vt_view[t][:, :, C:2 * C],
                          in_=vt_sb[:].rearrange("p (c ch) -> p c ch", c=8))
        if t % 2 == 1:
            r = t // 2
            nc.vector.tensor_tensor(
                out=keys_aug[64:128, bass.ts(r, NQR)],
                in0=keys_aug[64:128, bass.ts(r, NQR)],
                in1=keys_aug[64:128, bass.ts(r, NQR)],
                op=mybir.AluOpType.mult)
            for c in (2 * r + 2, 2 * r + 3):
                if c < NCH:
                    for pb in packed2:
                        nc.gpsimd.dma_start(out=pb[:, c * CH:(c + 1) * CH],
                                            in_=pkt[:, c * CH:(c + 1) * CH])

    # per-query bias for all tiles: bias_all[:, t] = -(q.mbar - c0 + kappa*|q|)
    ps_ball = ps_misc.tile([128, 512], F32, tag="misc", name="psball" + uniq)
    for qt in range(NQT):
        nc.tensor.matmul(ps_ball[:, qt:qt + 1], lhsT=q_aug[:, bass.ts(qt, 128)],
                         rhs=mb_sb[:], start=True, stop=True)
        nc.tensor.matmul(ps_ball[:, NQT + qt:NQT + qt + 1],
                         lhsT=q2[:, bass.ts(qt, 128)], rhs=ones64[:],
                         start=True, stop=True)
    qn_all = big.tile([128, NQT], F32, tag="qn_all")
    nc.scalar.activation(qn_all[:], ps_ball[:, NQT:2 * NQT],
                         mybir.ActivationFunctionType.Sqrt, scale=1.0)
    bias_all = big.tile([128, NQT], F32, tag="bias_all")
    nc.vector.scalar_tensor_tensor(
        out=bias_all[:], in0=qn_all[:], scalar=-BIAS_KAPPA,
        in1=ps_ball[:, 0:NQT],
        op0=mybir.AluOpType.mult, op1=mybir.AluOpType.add)

    ut_sb = big.tile([128, NQT * C], F16, tag="ut")
    for h in range(2):
        ps_ut = ps_misc.tile([128, 512], F32, tag="misc")
        for j in range(8):
            nc.tensor.matmul(ps_ut[:, j * C:(j + 1) * C],
                             lhsT=q_aug[0:64, bass.ts(8 * h + j, 128)],
                             rhs=wat_sb[:], start=True, stop=True)
        nc.scalar.copy(ut_sb[:, h * 512:(h + 1) * 512], ps_ut[:])

    stats_ps = ps_stat.tile([128, 512], F32, tag="stat")

    # ---- per-tile tail, split so every engine's inputs are a full
    # iteration old when its queue reaches them ----
    # tailP(t): Pool A = G + u, A2 = A^2 (emitted FIRST in the iteration so
    #           they precede the gathers in Pool's in-order queue)
    # tailC(t): PE stats matmuls + DVE max-over-K fold + out DMA (reads
    #           A/A2 produced one iteration earlier)
    def tailP(tq):
        G = G_tiles.pop(tq)
        ut_qt = ut_sb[:, bass.ts(tq, C)]
        A = tpool.tile([128, K * C], F16, tag="A")
        nc.vector.tensor_tensor(
            out=A[:].rearrange("p (k o) -> p k o", k=K),
            in0=G[:].rearrange("p (k o) -> p k o", k=K)[:, :, 0:C],
            in1=ut_qt.unsqueeze(1).broadcast_to([128, K, C]),
            op=mybir.AluOpType.add)
        A_tiles[tq] = A

    def tailQ(tq):
        A = A_tiles[tq]
        A2 = tpool.tile([128, K * C], F16, tag="A2")
        # last tiles' squares land in the drain, where DVE is idle but Pool
        # still has gathers queued - run them on DVE there
        eng = nc.vector if tq >= NQT - 4 else nc.gpsimd
        eng.tensor_tensor(out=A2[:], in0=A[:], in1=A[:],
                          op=mybir.AluOpType.mult)
        A2_tiles[tq] = A2

    def tailC(tq):
        A = A_tiles.pop(tq)
        A2 = A2_tiles.pop(tq)
        st, sp = (tq == 0), (tq == NQT - 1)
        nc.tensor.matmul(stats_ps[0:1, 0:H], lhsT=ones128[:], rhs=A[:, 0:H],
                         start=st, stop=sp, tile_position=(0, 0))
        nc.tensor.matmul(stats_ps[32:33, 0:H], lhsT=ones128[:], rhs=A[:, H:],
                         start=st, stop=sp, tile_position=(0, 32))
        nc.tensor.matmul(stats_ps[64:65, 0:H], lhsT=ones128[:], rhs=A2[:, 0:H],
                         start=st, stop=sp, tile_position=(0, 64))
        nc.tensor.matmul(stats_ps[96:97, 0:H], lhsT=ones128[:], rhs=A2[:, H:],
                         start=st, stop=sp, tile_position=(0, 96))
        # max over K: fold tree (8,4,2,1) on DVE fp16 (2x path; Pool has no
        # ISA max, and contiguous halves keep the 2-byte fast mode)
        M1 = tpool.tile([128, 8 * C], F16, tag="M1")
        nc.vector.tensor_tensor(out=M1[:], in0=A[:, 0:8 * C], in1=A[:, 8 * C:],
                                op=mybir.AluOpType.max)
        M2 = tpool.tile([128, 4 * C], F16, tag="M2")
        nc.vector.tensor_tensor(out=M2[:], in0=M1[:, 0:4 * C], in1=M1[:, 4 * C:],
                                op=mybir.AluOpType.max)
        M3 = tpool.tile([128, 2 * C], F16, tag="M3")
        nc.vector.tensor_tensor(out=M3[:], in0=M2[:, 0:2 * C], in1=M2[:, 2 * C:],
                                op=mybir.AluOpType.max)
        O = tpool.tile([128, C], F16, tag="O")
        nc.vector.tensor_tensor(out=O[:], in0=M3[:, 0:C], in1=M3[:, C:],
                                op=mybir.AluOpType.max)
        # out DMA rides SP, whose queue has nothing else in steady state,
        # so its wait on O can't block any other work.
        nc.sync.dma_start(out=out_o[bass.ts(tq, 128), :], in_=O[:])

    # ---- main loop over query tiles (software pipelined) ----
    # stage A(t): scores -> packed -> L1/L2 top-16 -> idxf
    # stage B(t): idx relayout (PE transposes) -> gtidx -> dma_gather
    G_tiles = {}
    idx_tiles = {}
    Xsb_tiles = {}
    A_tiles = {}
    A2_tiles = {}

    def stageA(qt):
        qcols = bass.ts(qt, 128)
        pk = packed2[qt % 2]
        pk_hi = pk[:].bitcast(F16).rearrange("p (n two) -> p n two", two=2)

        # scores -> packed fp16-high-lane (+bias) -> L1 top-8 per chunk
        Ct = tk.tile([128, CAND], F32, tag="C")
        for c in range(NCH):
            ps_s = ps_score.tile([128, CH], F32, tag="score",
                                 name=f"pss{uniq}_{qt}_{c}")
            nc.tensor.matmul(ps_s[:, 0:512], lhsT=q_aug[:, qcols],
                             rhs=keys_aug[:, c * CH:c * CH + 512],
                             start=True, stop=True)
            nc.tensor.matmul(ps_s[:, 512:1024], lhsT=q_aug[:, qcols],
                             rhs=keys_aug[:, c * CH + 512:(c + 1) * CH],
                             start=True, stop=True)
            nc.scalar.activation(pk_hi[:, c * CH:(c + 1) * CH, 1:2], ps_s[:],
                                 mybir.ActivationFunctionType.Identity,
                                 bias=bias_all[:, qt:qt + 1], scale=1.0)
            nc.vector.max(out=Ct[:, bass.ts(c, 8)],
                          in_=pk[:, c * CH:(c + 1) * CH])

        # L2: top-16 packed words
        T12 = tk.tile([128, K], F32, tag="T12")
        Cmr = tk.tile([128, CAND], F32, tag="Cmr")
        nc.vector.max(out=T12[:, 0:8], in_=Ct[:])
        nc.vector.match_replace(out=Cmr[:], in_to_replace=T12[:, 0:8],
                                in_values=Ct[:], imm_value=-1e30)
        nc.vector.max(out=T12[:, 8:16], in_=Cmr[:])
        idxf = tk.tile([128, K], F32, tag="idxf")
        nc.vector.tensor_copy(
            out=idxf[:].unsqueeze(2),
            in_=T12[:].bitcast(U16).rearrange("p (n two) -> p n two",
                                              two=2)[:, :, 0:1])
        idx_tiles[qt] = idxf

    def stageB1(qt):
        idxf = idx_tiles.pop(qt)
        # relayout idx [128q, 16k] -> wrap layout [16, 128]:
        # X[k, q] = idx[q, k] (one full transpose).  PE transpose runs after
        # this iteration's score matmuls; the Act copy sits after this
        # iteration's evacs, by which time ps_x is long done.
        ps_x = ps_misc.tile([128, 512], F32, tag="misc", name=f"psx{uniq}_{qt}")
        nc.tensor.transpose(ps_x[0:16, 0:128], idxf[:], identity[:])
        Xsb = tk.tile([16, 128], F32, tag="Xsb")
        nc.scalar.copy(Xsb[:], ps_x[0:16, 0:128])
        Xsb_tiles[qt] = Xsb

    def stageB2(qt):
        Xsb = Xsb_tiles.pop(qt)
        # per-16-block transposes ps_y[b, 16a+k] = X[k, 16a+b] (all
        # base-partition 0).  The block transposes take a free-duplicated
        # input so the output lands twice (partitions 0:16 and 16:32) - the
        # gather's tx Q7 core reads the index rows from partitions 16:32.
        ps_y = ps_misc.tile([128, 512], F32, tag="misc", name=f"psy{uniq}_{qt}")
        for a in range(8):
            nc.tensor.transpose(ps_y[0:16, 16 * a:16 * (a + 1)],
                                Xsb[:, 16 * a:16 * (a + 1)],
                                identity[0:16, 0:16])
        gt = gtidx2[qt % 2]
        # gtidx[b, 8k+a] = ps_y[b, 16a+k]; the gather's tx Q7 core reads the
        # index rows from partitions 16:32, so DMA-replicate them there.
        nc.scalar.activation(
            gt[0:16, :],
            ps_y[0:16, 0:128].rearrange("p (a k) -> p k a", a=8),
            mybir.ActivationFunctionType.Copy)
        # replicate rides the Act queue right after the copy that feeds it
        nc.scalar.dma_start(out=gt[16:32, :], in_=gt[0:16, :])

        # gather neighbor features G[q, k, 0:C] = v^T[nn[q, k], :]
        # (4 ops of 512 idxs - the Q7 idx scratch caps num_idxs at 512;
        #  f16 rows padded to 256 bytes to satisfy the descriptor-size rule)
        G = gpool.tile([128, K * 2 * C], F16, tag="G")
        Gv = G[:].rearrange("p (k o) -> p k o", k=K)
        for g in range(4):
            nc.gpsimd.dma_gather(
                out_ap=Gv[:, 4 * g:4 * (g + 1), :],
                in_ap=vt_dram,
                idxs_ap=gt[:, 32 * g:32 * (g + 1)],
                num_idxs=512,
                num_idxs_reg=512,
                elem_size=2 * C,
            )
        G_tiles[qt] = G

    # Virtual-iteration schedule.  Stage offsets (it = virtual iteration):
    #   stageA(it)     scores/evac/L1+L2 top-k   PE/Act/DVE
    #   stageB1(it-2)  idx transpose + Xsb       PE tail / Act tail
    #   stageB2(it-3)  blocks + gt + gathers     PE / Act tail / Pool tail
    #   tailP(it-4)    A = G+u, A2               Pool head
    #   tailC(it-6)    stats + fold + out        PE head / DVE head / SP
    # All cross-engine inputs are >= 1 iteration old except the intended
    # mm -> evac -> Max chunk pipeline and the same-iteration PE -> Act
    # relay (PE runs those ~4us before Act reaches them).
    SL = cfg.get("stage_limit", 3)
    TC = TD + 2
    for it in range(NQT + TC):
        if SL >= 3 and 0 <= it - TC:
            tailC(it - TC)
        if SL >= 3 and 0 <= it - TD - 1 < NQT:
            tailQ(it - TD - 1)
        if SL >= 3 and 0 <= it - TD < NQT:
            tailP(it - TD)
        if it < NQT:
            stageA(it)
        if SL >= 2 and 0 <= it - DB < NQT:
            stageB1(it - DB)
        if SL >= 2 and 0 <= it - DB - 1 < NQT:
            stageB2(it - DB - 1)

    # ---- epilogue: stats psum -> sbuf -> dram ----
    if SL >= 3:
        s_sb = big.tile([128, 512], F32, tag="s_sb")
        nc.vector.memset(s_sb[:], 0.0)
        for p in (0, 32, 64, 96):
            nc.scalar.copy(s_sb[p:p + 1, :], stats_ps[p:p + 1, :])
        nc.sync.dma_start(out=out_s, in_=s_sb[:])
    ctx.close()


def build_program(cfg, num_cores=8, reps=1):
    nc = bacc.Bacc("TRN2", target_bir_lowering=False, debug=False,
                   enable_asserts=False, num_devices=num_cores)
    C, NK, NQ = cfg["C"], cfg["NK"], cfg["NQ"]
    ins = {
        "f": nc.dram_tensor("f", [C, NK], F16, kind="ExternalInput").ap(),
        "fq": nc.dram_tensor("fq", [C, NQ], F16, kind="ExternalInput").ap(),
        "w2t": nc.dram_tensor("w2t", [C, C], F16, kind="ExternalInput").ap(),
        "wat": nc.dram_tensor("wat", [C, C], F16, kind="ExternalInput").ap(),
        "mb": nc.dram_tensor("mb", [128, 1], F16, kind="ExternalInput").ap(),
        "pkt": nc.dram_tensor("pkt", [128, NK], F32, kind="ExternalInput").ap(),
    }
    outs = {
        "out_o": nc.dram_tensor("out_o", [NQ, C], F16,
                                kind="ExternalOutput").ap(),
        "out_s": nc.dram_tensor("out_s", [128, 512], F32,
                                kind="ExternalOutput").ap(),
    }
    with tile.TileContext(nc) as tc:
        for r in range(reps):
            emit(tc, ins, outs, cfg, uniq=f"_r{r}")
    nc.compile()
    return nc


_PROGRAM_CACHE = {}


def get_program(num_cores=8):
    key = num_cores
    if key not in _PROGRAM_CACHE:
        _PROGRAM_CACHE[key] = build_program(default_cfg(), num_cores)
    return _PROGRAM_CACHE[key]


def make_in_maps(x, W, n_cores=8):
    """Build the per-core input dicts from the full inputs."""
    cfg = default_cfg()
    C, NQ = cfg["C"], cfg["NQ"]
    B, _, N, _ = x.shape
    per_batch = N // NQ
    f16 = np.ascontiguousarray(x[:, :, :, 0]).astype(np.float16)  # (B, C, N)
    W16 = W.astype(np.float16)
    W1, W2 = W16[:, :C], W16[:, C:]
    w2t = np.ascontiguousarray(W2.T)
    wat = np.ascontiguousarray((W1 - W2).T)
    pkt = np.broadcast_to(
        np.arange(N, dtype=np.uint32)[None, :], (128, N)).copy().view(np.float32)
    mbs = []
    for b in range(B):
        fb = f16[b].astype(np.float32)
        mbar = fb.mean(axis=1)                      # (C,)
        c0 = float(0.5 * (fb * fb).sum(axis=0).mean())
        mbv = np.zeros((128, 1), np.float16)
        mbv[0:C, 0] = (-mbar).astype(np.float16)
        # contracted against the -0.5 rows of q_aug: 64 * (-0.5) * (-c0/32) = c0
        mbv[C:128, 0] = np.float16(-c0 / 32.0)
        mbs.append(mbv)
    in_maps = []
    for c in range(n_cores):
        b, qb = c // per_batch, c % per_batch
        in_maps.append({
            "f": np.ascontiguousarray(f16[b]),
            "fq": np.ascontiguousarray(f16[b][:, qb * NQ:(qb + 1) * NQ]),
            "w2t": w2t,
            "wat": wat,
            "mb": mbs[b],
            "pkt": pkt,
        })
    return in_maps


def host_epilogue(m_full, s1, s2, gamma, beta, count):
    mean = s1 / count
    var = s2 / count - mean * mean
    a = gamma.astype(np.float64) / np.sqrt(var + BN_EPS)
    b = beta.astype(np.float64) - a * mean
    y = a[None, :, None] * m_full.astype(np.float64) + b[None, :, None]
    y = np.where(y >= 0, y, LRELU_SLOPE * y)
    return y.astype(np.float32)


def kernel(x, W, gamma, beta):
    """Full (unsharded) inputs -> full output. See module docstring."""
    from concourse import bass_utils

    x = np.asarray(x)
    W = np.asarray(W)
    gamma = np.asarray(gamma)
    beta = np.asarray(beta)

    B, C, N, _ = x.shape
    K = 16
    assert (B, C, N) == (2, 64, 8192), "kernel hardcoded for this problem size"

    cfg = default_cfg()
    NQ = cfg["NQ"]
    n_cores = 8
    per_batch = N // NQ

    in_maps = make_in_maps(x, W, n_cores)
    nc = get_program(n_cores)
    res = bass_utils.run_bass_kernel_spmd(nc, in_maps, list(range(n_cores)))
    results = res.results

    m_full = np.empty((B, C, N), np.float32)
    s1 = np.zeros(C, np.float64)
    s2 = np.zeros(C, np.float64)
    H = K * C // 2
    for c in range(n_cores):
        b, qb = c // per_batch, c % per_batch
        m_full[b, :, qb * NQ:(qb + 1) * NQ] = \
            results[c]["out_o"].astype(np.float32).T
        st = results[c]["out_s"].astype(np.float64)
        s1 += (st[0, :H].reshape(K // 2, C) + st[32, :H].reshape(K // 2, C)).sum(0)
        s2 += (st[64, :H].reshape(K // 2, C) + st[96, :H].reshape(K // 2, C)).sum(0)

    count = float(B) * N * K
    return host_epilogue(m_full, s1, s2, gamma, beta, count)


if __name__ == "__main__":
    sys.path.insert(0, os.path.dirname(os.path.abspath(__file__)))
    import reference

    inputs = {k: np.asarray(v) for k, v in reference.setup_inputs().items()}
    out = kernel(**inputs)
    import jax
    cpu = jax.devices("cpu")[0]
    with jax.default_device(cpu):
        exp = np.asarray(reference.reference(
            **{k: jax.device_put(v, cpu) for k, v in inputs.items()}))
    err = np.abs(out - exp)
    rel = np.linalg.norm(out - exp) / np.linalg.norm(exp)
    print("max abs err:", err.max(), "rel l2 err:", rel)



# revision 29
# speedup vs baseline: 1.8105x; 1.2315x over previous
"""Trainium2 Bass kernel for nn_DynConv2d (DGCNN EdgeConv layer).

Reference computation (B=2, C=64, N=8192, K=16, C_out=64):
  f = x[:,:,:,0]
  nn_idx = top-16 nearest neighbors by squared L2 over point features
  feat = concat([x_i, x_j - x_i])          # (B, 2C, N, K)
  y = W @ feat                             # 1x1 conv
  y = BatchNorm2d(y)  (training stats over (B,N,K))
  y = LeakyReLU(0.2)(y)
  out = max over K                         # (B, C_out, N)

Algebraic restructuring:
  * W @ [x_i; x_j - x_i] = u[:,i] + v[:,j] with u = (W1-W2)@f, v = W2@f.
  * BN+LeakyReLU is per-channel monotone, so max over K commutes; the kernel
    returns max_k(u+v_j) plus the BN batch stats (sum / sum-sq); the final
    affine + lrelu runs on host.
  * KNN score s = q.m - |m|^2/2 via a 128-contraction fp16 matmul: rows 0:64
    are features, rows 64:128 of keys hold -f^2/2 (q side holds ones), so no
    separate |m|^2 row materialization is needed.

Top-16 selection (per query row of 8192 fp32 PSUM scores):
  * Scalar engine evacuates PSUM -> SBUF converting to fp16 *into the high
    u16 lane* of a packed fp32 word whose low u16 lane is the key index
    (preloaded from a host template).  Numeric fp32 ordering of the packed
    word == lexicographic (fp16 score, index) ordering, so a single DVE
    Max top-8 per 1024-chunk yields values *and* indices in one pass - no
    MaxIndex, no scatter.
  * A per-query bias (-(q.mbar - c0 + 3.9|q|)) is added during evacuation to
    center the interesting (top-16) scores near zero, which shrinks the fp16
    rounding error where it matters.  Any per-query constant preserves the
    within-row order, so this never breaks correctness.
  * L2: Max + MatchReplace + Max over the 64 chunk-candidates -> top-16
    packed words; low lanes are the global key indices.
  * Indices are relayed out to the dma_gather wrap layout (16 partitions,
    idx[n%16, n//16] = nn-index of slot n = k*128+q) with 16 tiny 16x16 PE
    transposes, then one InstDMAGatherAnt fetches all 2048 neighbor rows of
    v^T per query tile.

Sharding: 8 cores; core c handles batch c//4, query block c%4 (2048 queries),
against all 8192 keys of its batch.
"""

import os
import sys

import numpy as np

sys.path.insert(0, "/opt/trn_rl_repo")

import concourse.bacc as bacc
import concourse.bass as bass
import concourse.mybir as mybir
import concourse.tile as tile
from concourse.masks import make_identity

F32 = mybir.dt.float32
F16 = mybir.dt.float16
U16 = mybir.dt.uint16
I16 = mybir.dt.int16

BN_EPS = 1e-5
LRELU_SLOPE = 0.2
BIAS_KAPPA = 3.9


def default_cfg():
    return dict(C=64, NK=8192, NQ=2048, K=16, CH=1024, B_DIST=1,
                TAIL_DIST=4)


def emit(tc, ins, outs, cfg, uniq=""):
    """Per-core program.

    ins:  f (64, NK) f16, fq (64, NQ) f16 (this core's query slice),
          w2t (C, C) f16, wat (C, C) f16,
          mb (128, 1) f16  [rows 0:64 = -mean_keys(f), rows 64:128 = c0/64],
          pkt (128, NK) f32 [u32 words: low u16 = column index, high = 0]
    outs: out_o (NQ, C) f16   max-over-K of u+v (pre-BN), query-major
          out_s (128, 512) f32  rows {0,32,64,96} = psum stats
    """
    nc = tc.nc
    C = cfg["C"]          # 64
    NK = cfg["NK"]        # 8192
    NQ = cfg["NQ"]        # 2048
    K = cfg["K"]          # 16
    CH = cfg["CH"]        # 1024  L1 top-8 chunk
    DB = cfg.get("B_DIST", 2)     # stageB(t-DB): idx relay + gather
    TD = cfg.get("TAIL_DIST", 4)  # tail(t-TD): A=G+u, stats, max, out
    # Every cross-engine dependency is >= 1 full iteration stale, so each
    # in-order engine queue runs back-to-back at its own pace and the
    # period is set by the busiest engine (DVE), not by the
    # Max->L2->transpose->gt->gather->add->stats relay chain.
    NQT = NQ // 128       # 16 query tiles
    NCH = NK // CH        # 8 chunks
    CAND = 8 * NCH        # 64 candidates
    H = K * C // 2        # 512 (stats half-width)

    f, fq, w2t, wat, mb, pkt = (ins["f"], ins["fq"], ins["w2t"], ins["wat"],
                                ins["mb"], ins["pkt"])
    out_o, out_s = outs["out_o"], outs["out_s"]

    from contextlib import ExitStack
    ctx = ExitStack()
    dram_pool = ctx.enter_context(tc.tile_pool(name="dram" + uniq, bufs=1,
                                               space="DRAM"))
    vt_dram = dram_pool.tile([NK, 2 * C], F16, tag="vt", name="vt_t" + uniq)[:]

    cpool = ctx.enter_context(tc.tile_pool(name="consts" + uniq, bufs=1))
    big = ctx.enter_context(tc.tile_pool(name="big" + uniq, bufs=1))
    tk = ctx.enter_context(tc.tile_pool(name="topk" + uniq, bufs=3))
    vpool = ctx.enter_context(tc.tile_pool(name="vstage" + uniq, bufs=8))
    gpool = ctx.enter_context(tc.tile_pool(name="gather" + uniq, bufs=4))
    tpool = ctx.enter_context(tc.tile_pool(name="tail" + uniq, bufs=4))
    ps_score = ctx.enter_context(tc.tile_pool(name="ps_score" + uniq, bufs=2,
                                              space="PSUM"))
    ps_stat = ctx.enter_context(tc.tile_pool(name="ps_stat" + uniq, bufs=1,
                                             space="PSUM"))
    ps_misc = ctx.enter_context(tc.tile_pool(name="ps_misc" + uniq, bufs=3,
                                             space="PSUM"))

    # ---- constants / inputs ----
    identity = cpool.tile([128, 128], F32, tag="ident")
    make_identity(nc, identity[:])
    ones64 = cpool.tile([64, 1], F16, tag="ones64")
    nc.vector.memset(ones64[:], 1.0)
    ones128 = cpool.tile([128, 1], F16, tag="ones128")
    nc.vector.memset(ones128[:], 1.0)
    w2t_sb = cpool.tile([128, C], F16, tag="w2t")
    wat_sb = cpool.tile([C, C], F16, tag="wat")
    mb_sb = cpool.tile([128, 1], F16, tag="mb")

    # keys_aug: rows 0:64 = f, rows 64:128 = -f^2/2 (built in place from a
    # second copy of f so every engine op stays partition-aligned).
    # DMAs are spread across engine queues so the prologue isn't serialized
    # on SP.  Prologue engine placement keeps the steady-state bottleneck
    # (DVE) light: f^2 squaring runs on DVE (idle in prologue, 2-byte 2x
    # path), v^T/u^T PSUM evacuation on Act.
    keys_aug = big.tile([128, NK], F16, tag="keys_aug")
    q_aug = big.tile([128, NQ], F16, tag="q_aug")
    NQR = NK // 4
    # keys2 r0 is the head of the prologue critical chain (vt matmuls ->
    # f^2 -> first score matmul); it leads the Act DMA queue
    for r in range(4):
        nc.scalar.dma_start(out=keys_aug[64:128, bass.ts(r, NQR)],
                            in_=f[:, bass.ts(r, NQR)])
    nc.sync.dma_start(out=q_aug[0:64, :], in_=fq)
    # w2t lives at base partition 64 to pair with keys_aug[64:128] in matmuls
    nc.sync.dma_start(out=w2t_sb[64:128, :], in_=w2t)
    nc.sync.dma_start(out=wat_sb[:], in_=wat)
    nc.sync.dma_start(out=mb_sb[:], in_=mb)
    # rows 64:128 hold -0.5 so the score matmul contracts -0.5 * f^2 rows
    nc.gpsimd.memset(q_aug[64:128, :], -0.5)

    # fq^2 for the per-query |q| bias estimate (DVE, 2-byte fast path)
    q2 = big.tile([64, NQ], F16, tag="q2")
    nc.vector.tensor_tensor(out=q2[:], in0=q_aug[0:64, :], in1=q_aug[0:64, :],
                            op=mybir.AluOpType.mult)

    # Single packed score buffer (low u16 lanes = key index template).  The
    # evac(t+1, c) -> Max(t, c) WAR is 8 chunk-slots stale, so one buffer
    # pipelines with no stalls.  keys1 quarters and template chunks
    # interleave on SP so chunk c's template lands just before Max(0, c).
    packed2 = [big.tile([128, NK], F32, tag=f"packed{i}",
                        name=f"packed{i}" + uniq) for i in range(2)]
    for r in range(4):
        nc.sync.dma_start(out=keys_aug[0:64, bass.ts(r, NQR)],
                          in_=f[:, bass.ts(r, NQR)])
    # gather index tiles (rows 16:128 must stay 0 for the executor's
    # bounds check; only rows 0:32 are consumed).  Two buffers so tile t's
    # index relay never waits for tile t-1's gathers to finish reading.
    gtidx2 = []
    for gi in range(2):
        gx = big.tile([128, 128], I16, tag=f"gtidx{gi}")
        nc.gpsimd.memset(gx[:], 0)
        gtidx2.append(gx)
    # index template rides the otherwise-idle Pool DMA queue; chunk c only
    # needs to land before Max(0, c), so only c0/c1 contend with the
    # critical keys transfers - the rest are issued after the vt loop
    for c in range(2):
        for pb in packed2:
            nc.gpsimd.dma_start(out=pb[:, c * CH:(c + 1) * CH],
                                in_=pkt[:, c * CH:(c + 1) * CH])

    # ---- v^T matmuls (read raw f in rows 64:128) + quartered -f^2/2 ----
    # Emission order per quarter: vt matmuls reading that quarter first, then
    # the in-place squaring of the same columns (readers before writer).
    vt_view = vt_dram.rearrange("(t c p) ch -> t p c ch", t=8, c=8)
    for t in range(8):
        ps_vt = ps_misc.tile([128, 512], F32, tag="misc")
        for c in range(8):
            nc.tensor.matmul(ps_vt[:, c * C:(c + 1) * C],
                             lhsT=keys_aug[64:128, bass.ts(8 * t + c, 128)],
                             rhs=w2t_sb[64:128, :], start=True, stop=True)
        vt_sb = vpool.tile([128, 512], F16, tag="vt_sb")
        nc.vector.tensor_copy(out=vt_sb[:], in_=ps_vt[:])
        nc.sync.dma_start(out=vt_view[t][:, :, 0:C],
                          in_=vt_sb[:].rearrange("p (c ch) -> p c ch", c=8))
        nc.sync.dma_start(out=vt_view[t][:, :, C:2 * C],
                          in_=vt_sb[:].rearrange("p (c ch) -> p c ch", c=8))
        if t % 2 == 1:
            r = t // 2
            nc.vector.tensor_tensor(
                out=keys_aug[64:128, bass.ts(r, NQR)],
                in0=keys_aug[64:128, bass.ts(r, NQR)],
                in1=keys_aug[64:128, bass.ts(r, NQR)],
                op=mybir.AluOpType.mult)
            for c in (2 * r + 2, 2 * r + 3):
                if c < NCH:
                    for pb in packed2:
                        nc.gpsimd.dma_start(out=pb[:, c * CH:(c + 1) * CH],
                                            in_=pkt[:, c * CH:(c + 1) * CH])

    # per-query bias for all tiles: bias_all[:, t] = -(q.mbar - c0 + kappa*|q|)
    ps_ball = ps_misc.tile([128, 512], F32, tag="misc", name="psball" + uniq)
    for qt in range(NQT):
        nc.tensor.matmul(ps_ball[:, qt:qt + 1], lhsT=q_aug[:, bass.ts(qt, 128)],
                         rhs=mb_sb[:], start=True, stop=True)
        nc.tensor.matmul(ps_ball[:, NQT + qt:NQT + qt + 1],
                         lhsT=q2[:, bass.ts(qt, 128)], rhs=ones64[:],
                         start=True, stop=True)
    qn_all = big.tile([128, NQT], F32, tag="qn_all")
    nc.scalar.activation(qn_all[:], ps_ball[:, NQT:2 * NQT],
                         mybir.ActivationFunctionType.Sqrt, scale=1.0)
    bias_all = big.tile([128, NQT], F32, tag="bias_all")
    nc.vector.scalar_tensor_tensor(
        out=bias_all[:], in0=qn_all[:], scalar=-BIAS_KAPPA,
        in1=ps_ball[:, 0:NQT],
        op0=mybir.AluOpType.mult, op1=mybir.AluOpType.add)

    ut_sb = big.tile([128, NQT * C], F16, tag="ut")
    for h in range(2):
        ps_ut = ps_misc.tile([128, 512], F32, tag="misc")
        for j in range(8):
            nc.tensor.matmul(ps_ut[:, j * C:(j + 1) * C],
                             lhsT=q_aug[0:64, bass.ts(8 * h + j, 128)],
                             rhs=wat_sb[:], start=True, stop=True)
        nc.scalar.copy(ut_sb[:, h * 512:(h + 1) * 512], ps_ut[:])

    stats_ps = ps_stat.tile([128, 512], F32, tag="stat")

    # ---- per-tile tail, split so every engine's inputs are a full
    # iteration old when its queue reaches them ----
    # tailP(t): Pool A = G + u, A2 = A^2 (emitted FIRST in the iteration so
    #           they precede the gathers in Pool's in-order queue)
    # tailC(t): PE stats matmuls + DVE max-over-K fold + out DMA (reads
    #           A/A2 produced one iteration earlier)
    def tailP(tq):
        G = G_tiles.pop(tq)
        ut_qt = ut_sb[:, bass.ts(tq, C)]
        A = tpool.tile([128, K * C], F16, tag="A")
        nc.vector.tensor_tensor(
            out=A[:].rearrange("p (k o) -> p k o", k=K),
            in0=G[:].rearrange("p (k o) -> p k o", k=K)[:, :, 0:C],
            in1=ut_qt.unsqueeze(1).broadcast_to([128, K, C]),
            op=mybir.AluOpType.add)
        A_tiles[tq] = A

    def tailQ(tq):
        A = A_tiles[tq]
        A2 = tpool.tile([128, K * C], F16, tag="A2")
        # Square on Act (same act-table set as Identity/Copy/Sqrt, so no
        # table reloads); keeps Pool at gathers only.  Last tiles land in
        # the drain where DVE is idle - run those there.
        if tq >= NQT - 4:
            nc.vector.tensor_tensor(out=A2[:], in0=A[:], in1=A[:],
                                    op=mybir.AluOpType.mult)
        else:
            nc.scalar.activation(A2[:], A[:],
                                 mybir.ActivationFunctionType.Square,
                                 bias=0.0, scale=1.0)
        A2_tiles[tq] = A2

    def tailC(tq):
        A = A_tiles.pop(tq)
        A2 = A2_tiles.pop(tq)
        st, sp = (tq == 0), (tq == NQT - 1)
        nc.tensor.matmul(stats_ps[0:1, 0:H], lhsT=ones128[:], rhs=A[:, 0:H],
                         start=st, stop=sp, tile_position=(0, 0))
        nc.tensor.matmul(stats_ps[32:33, 0:H], lhsT=ones128[:], rhs=A[:, H:],
                         start=st, stop=sp, tile_position=(0, 32))
        nc.tensor.matmul(stats_ps[64:65, 0:H], lhsT=ones128[:], rhs=A2[:, 0:H],
                         start=st, stop=sp, tile_position=(0, 64))
        nc.tensor.matmul(stats_ps[96:97, 0:H], lhsT=ones128[:], rhs=A2[:, H:],
                         start=st, stop=sp, tile_position=(0, 96))
        # max over K: fold tree (8,4,2,1) on DVE fp16 (2x path; Pool has no
        # ISA max, and contiguous halves keep the 2-byte fast mode)
        M1 = tpool.tile([128, 8 * C], F16, tag="M1")
        nc.vector.tensor_tensor(out=M1[:], in0=A[:, 0:8 * C], in1=A[:, 8 * C:],
                                op=mybir.AluOpType.max)
        M2 = tpool.tile([128, 4 * C], F16, tag="M2")
        nc.vector.tensor_tensor(out=M2[:], in0=M1[:, 0:4 * C], in1=M1[:, 4 * C:],
                                op=mybir.AluOpType.max)
        M3 = tpool.tile([128, 2 * C], F16, tag="M3")
        nc.vector.tensor_tensor(out=M3[:], in0=M2[:, 0:2 * C], in1=M2[:, 2 * C:],
                                op=mybir.AluOpType.max)
        O = tpool.tile([128, C], F16, tag="O")
        nc.vector.tensor_tensor(out=O[:], in0=M3[:, 0:C], in1=M3[:, C:],
                                op=mybir.AluOpType.max)
        # out DMA rides SP, whose queue has nothing else in steady state,
        # so its wait on O can't block any other work.
        nc.sync.dma_start(out=out_o[bass.ts(tq, 128), :], in_=O[:])

    # ---- main loop over query tiles (software pipelined) ----
    # stage A(t): scores -> packed -> L1/L2 top-16 -> idxf
    # stage B(t): idx relayout (PE transposes) -> gtidx -> dma_gather
    G_tiles = {}
    idx_tiles = {}
    Xsb_tiles = {}
    A_tiles = {}
    A2_tiles = {}

    def stageA(qt):
        qcols = bass.ts(qt, 128)
        pk = packed2[qt % 2]
        pk_hi = pk[:].bitcast(F16).rearrange("p (n two) -> p n two", two=2)

        # scores -> packed fp16-high-lane (+bias) -> L1 top-8 per chunk
        Ct = tk.tile([128, CAND], F32, tag="C")
        for c in range(NCH):
            ps_s = ps_score.tile([128, CH], F32, tag="score",
                                 name=f"pss{uniq}_{qt}_{c}")
            nc.tensor.matmul(ps_s[:, 0:512], lhsT=q_aug[:, qcols],
                             rhs=keys_aug[:, c * CH:c * CH + 512],
                             start=True, stop=True)
            nc.tensor.matmul(ps_s[:, 512:1024], lhsT=q_aug[:, qcols],
                             rhs=keys_aug[:, c * CH + 512:(c + 1) * CH],
                             start=True, stop=True)
            nc.scalar.activation(pk_hi[:, c * CH:(c + 1) * CH, 1:2], ps_s[:],
                                 mybir.ActivationFunctionType.Identity,
                                 bias=bias_all[:, qt:qt + 1], scale=1.0)
            nc.vector.max(out=Ct[:, bass.ts(c, 8)],
                          in_=pk[:, c * CH:(c + 1) * CH])

        # L2: top-16 packed words
        T12 = tk.tile([128, K], F32, tag="T12")
        Cmr = tk.tile([128, CAND], F32, tag="Cmr")
        nc.vector.max(out=T12[:, 0:8], in_=Ct[:])
        nc.vector.match_replace(out=Cmr[:], in_to_replace=T12[:, 0:8],
                                in_values=Ct[:], imm_value=-1e30)
        nc.vector.max(out=T12[:, 8:16], in_=Cmr[:])
        idxf = tk.tile([128, K], F32, tag="idxf")
        nc.vector.tensor_copy(
            out=idxf[:].unsqueeze(2),
            in_=T12[:].bitcast(U16).rearrange("p (n two) -> p n two",
                                              two=2)[:, :, 0:1])
        idx_tiles[qt] = idxf

    def stageB1(qt):
        idxf = idx_tiles.pop(qt)
        # relayout idx [128q, 16k] -> wrap layout [16, 128]:
        # X[k, q] = idx[q, k] (one full transpose).  PE transpose runs after
        # this iteration's score matmuls; the Act copy sits after this
        # iteration's evacs, by which time ps_x is long done.
        ps_x = ps_misc.tile([128, 512], F32, tag="misc", name=f"psx{uniq}_{qt}")
        nc.tensor.transpose(ps_x[0:16, 0:128], idxf[:], identity[:])
        Xsb = tk.tile([16, 128], F32, tag="Xsb")
        nc.scalar.copy(Xsb[:], ps_x[0:16, 0:128])
        Xsb_tiles[qt] = Xsb

    def stageB2(qt):
        Xsb = Xsb_tiles.pop(qt)
        # per-16-block transposes ps_y[b, 16a+k] = X[k, 16a+b] (all
        # base-partition 0).  The block transposes take a free-duplicated
        # input so the output lands twice (partitions 0:16 and 16:32) - the
        # gather's tx Q7 core reads the index rows from partitions 16:32.
        ps_y = ps_misc.tile([128, 512], F32, tag="misc", name=f"psy{uniq}_{qt}")
        for a in range(8):
            nc.tensor.transpose(ps_y[0:16, 16 * a:16 * (a + 1)],
                                Xsb[:, 16 * a:16 * (a + 1)],
                                identity[0:16, 0:16])
        gt = gtidx2[qt % 2]
        # gtidx[b, 8k+a] = ps_y[b, 16a+k]; the gather's tx Q7 core reads the
        # index rows from partitions 16:32, so DMA-replicate them there.
        nc.scalar.activation(
            gt[0:16, :],
            ps_y[0:16, 0:128].rearrange("p (a k) -> p k a", a=8),
            mybir.ActivationFunctionType.Copy)
        # replicate rides the Act queue right after the copy that feeds it
        nc.scalar.dma_start(out=gt[16:32, :], in_=gt[0:16, :])

        # gather neighbor features G[q, k, 0:C] = v^T[nn[q, k], :]
        # (4 ops of 512 idxs - the Q7 idx scratch caps num_idxs at 512;
        #  f16 rows padded to 256 bytes to satisfy the descriptor-size rule)
        G = gpool.tile([128, K * 2 * C], F16, tag="G")
        Gv = G[:].rearrange("p (k o) -> p k o", k=K)
        for g in range(4):
            nc.gpsimd.dma_gather(
                out_ap=Gv[:, 4 * g:4 * (g + 1), :],
                in_ap=vt_dram,
                idxs_ap=gt[:, 32 * g:32 * (g + 1)],
                num_idxs=512,
                num_idxs_reg=512,
                elem_size=2 * C,
            )
        G_tiles[qt] = G

    # Virtual-iteration schedule.  Stage offsets (it = virtual iteration):
    #   stageA(it)     scores/evac/L1+L2 top-k   PE/Act/DVE
    #   stageB1(it-2)  idx transpose + Xsb       PE tail / Act tail
    #   stageB2(it-3)  blocks + gt + gathers     PE / Act tail / Pool tail
    #   tailP(it-4)    A = G+u, A2               Pool head
    #   tailC(it-6)    stats + fold + out        PE head / DVE head / SP
    # All cross-engine inputs are >= 1 iteration old except the intended
    # mm -> evac -> Max chunk pipeline and the same-iteration PE -> Act
    # relay (PE runs those ~4us before Act reaches them).
    SL = cfg.get("stage_limit", 3)
    TC = TD + 2
    for it in range(NQT + TC):
        if SL >= 3 and 0 <= it - TC:
            tailC(it - TC)
        if SL >= 3 and 0 <= it - TD - 1 < NQT:
            tailQ(it - TD - 1)
        if SL >= 3 and 0 <= it - TD < NQT:
            tailP(it - TD)
        if it < NQT:
            stageA(it)
        if SL >= 2 and 0 <= it - DB < NQT:
            stageB1(it - DB)
        if SL >= 2 and 0 <= it - DB - 1 < NQT:
            stageB2(it - DB - 1)

    # ---- epilogue: stats psum -> sbuf -> dram ----
    if SL >= 3:
        s_sb = big.tile([128, 512], F32, tag="s_sb")
        nc.vector.memset(s_sb[:], 0.0)
        for p in (0, 32, 64, 96):
            nc.scalar.copy(s_sb[p:p + 1, :], stats_ps[p:p + 1, :])
        nc.sync.dma_start(out=out_s, in_=s_sb[:])
    ctx.close()


def build_program(cfg, num_cores=8, reps=1):
    nc = bacc.Bacc("TRN2", target_bir_lowering=False, debug=False,
                   enable_asserts=False, num_devices=num_cores)
    C, NK, NQ = cfg["C"], cfg["NK"], cfg["NQ"]
    ins = {
        "f": nc.dram_tensor("f", [C, NK], F16, kind="ExternalInput").ap(),
        "fq": nc.dram_tensor("fq", [C, NQ], F16, kind="ExternalInput").ap(),
        "w2t": nc.dram_tensor("w2t", [C, C], F16, kind="ExternalInput").ap(),
        "wat": nc.dram_tensor("wat", [C, C], F16, kind="ExternalInput").ap(),
        "mb": nc.dram_tensor("mb", [128, 1], F16, kind="ExternalInput").ap(),
        "pkt": nc.dram_tensor("pkt", [128, NK], F32, kind="ExternalInput").ap(),
    }
    outs = {
        "out_o": nc.dram_tensor("out_o", [NQ, C], F16,
                                kind="ExternalOutput").ap(),
        "out_s": nc.dram_tensor("out_s", [128, 512], F32,
                                kind="ExternalOutput").ap(),
    }
    with tile.TileContext(nc) as tc:
        for r in range(reps):
            emit(tc, ins, outs, cfg, uniq=f"_r{r}")
    nc.compile()
    return nc


_PROGRAM_CACHE = {}


def get_program(num_cores=8):
    key = num_cores
    if key not in _PROGRAM_CACHE:
        _PROGRAM_CACHE[key] = build_program(default_cfg(), num_cores)
    return _PROGRAM_CACHE[key]


def make_in_maps(x, W, n_cores=8):
    """Build the per-core input dicts from the full inputs."""
    cfg = default_cfg()
    C, NQ = cfg["C"], cfg["NQ"]
    B, _, N, _ = x.shape
    per_batch = N // NQ
    f16 = np.ascontiguousarray(x[:, :, :, 0]).astype(np.float16)  # (B, C, N)
    W16 = W.astype(np.float16)
    W1, W2 = W16[:, :C], W16[:, C:]
    w2t = np.ascontiguousarray(W2.T)
    wat = np.ascontiguousarray((W1 - W2).T)
    pkt = np.broadcast_to(
        np.arange(N, dtype=np.uint32)[None, :], (128, N)).copy().view(np.float32)
    mbs = []
    for b in range(B):
        fb = f16[b].astype(np.float32)
        mbar = fb.mean(axis=1)                      # (C,)
        c0 = float(0.5 * (fb * fb).sum(axis=0).mean())
        mbv = np.zeros((128, 1), np.float16)
        mbv[0:C, 0] = (-mbar).astype(np.float16)
        # contracted against the -0.5 rows of q_aug: 64 * (-0.5) * (-c0/32) = c0
        mbv[C:128, 0] = np.float16(-c0 / 32.0)
        mbs.append(mbv)
    in_maps = []
    for c in range(n_cores):
        b, qb = c // per_batch, c % per_batch
        in_maps.append({
            "f": np.ascontiguousarray(f16[b]),
            "fq": np.ascontiguousarray(f16[b][:, qb * NQ:(qb + 1) * NQ]),
            "w2t": w2t,
            "wat": wat,
            "mb": mbs[b],
            "pkt": pkt,
        })
    return in_maps


def host_epilogue(m_full, s1, s2, gamma, beta, count):
    mean = s1 / count
    var = s2 / count - mean * mean
    a = gamma.astype(np.float64) / np.sqrt(var + BN_EPS)
    b = beta.astype(np.float64) - a * mean
    y = a[None, :, None] * m_full.astype(np.float64) + b[None, :, None]
    y = np.where(y >= 0, y, LRELU_SLOPE * y)
    return y.astype(np.float32)


def kernel(x, W, gamma, beta):
    """Full (unsharded) inputs -> full output. See module docstring."""
    from concourse import bass_utils

    x = np.asarray(x)
    W = np.asarray(W)
    gamma = np.asarray(gamma)
    beta = np.asarray(beta)

    B, C, N, _ = x.shape
    K = 16
    assert (B, C, N) == (2, 64, 8192), "kernel hardcoded for this problem size"

    cfg = default_cfg()
    NQ = cfg["NQ"]
    n_cores = 8
    per_batch = N // NQ

    in_maps = make_in_maps(x, W, n_cores)
    nc = get_program(n_cores)
    res = bass_utils.run_bass_kernel_spmd(nc, in_maps, list(range(n_cores)))
    results = res.results

    m_full = np.empty((B, C, N), np.float32)
    s1 = np.zeros(C, np.float64)
    s2 = np.zeros(C, np.float64)
    H = K * C // 2
    for c in range(n_cores):
        b, qb = c // per_batch, c % per_batch
        m_full[b, :, qb * NQ:(qb + 1) * NQ] = \
            results[c]["out_o"].astype(np.float32).T
        st = results[c]["out_s"].astype(np.float64)
        s1 += (st[0, :H].reshape(K // 2, C) + st[32, :H].reshape(K // 2, C)).sum(0)
        s2 += (st[64, :H].reshape(K // 2, C) + st[96, :H].reshape(K // 2, C)).sum(0)

    count = float(B) * N * K
    return host_epilogue(m_full, s1, s2, gamma, beta, count)


if __name__ == "__main__":
    sys.path.insert(0, os.path.dirname(os.path.abspath(__file__)))
    import reference

    inputs = {k: np.asarray(v) for k, v in reference.setup_inputs().items()}
    out = kernel(**inputs)
    import jax
    cpu = jax.devices("cpu")[0]
    with jax.default_device(cpu):
        exp = np.asarray(reference.reference(
            **{k: jax.device_put(v, cpu) for k, v in inputs.items()}))
    err = np.abs(out - exp)
    rel = np.linalg.norm(out - exp) / np.linalg.norm(exp)
    print("max abs err:", err.max(), "rel l2 err:", rel)



# revision 37
# speedup vs baseline: 1.8731x; 1.0346x over previous
"""Trainium2 Bass kernel for nn_DynConv2d (DGCNN EdgeConv layer).

Reference computation (B=2, C=64, N=8192, K=16, C_out=64):
  f = x[:,:,:,0]
  nn_idx = top-16 nearest neighbors by squared L2 over point features
  feat = concat([x_i, x_j - x_i])          # (B, 2C, N, K)
  y = W @ feat                             # 1x1 conv
  y = BatchNorm2d(y)  (training stats over (B,N,K))
  y = LeakyReLU(0.2)(y)
  out = max over K                         # (B, C_out, N)

Algebraic restructuring:
  * W @ [x_i; x_j - x_i] = u[:,i] + v[:,j] with u = (W1-W2)@f, v = W2@f.
  * BN+LeakyReLU is per-channel monotone, so max over K commutes; the kernel
    returns max_k(u+v_j) plus the BN batch stats (sum / sum-sq); the final
    affine + lrelu runs on host.
  * KNN score s = q.m - |m|^2/2 via a 128-contraction fp16 matmul: rows 0:64
    are features, rows 64:128 of keys hold -f^2/2 (q side holds ones), so no
    separate |m|^2 row materialization is needed.

Top-16 selection (per query row of 8192 fp32 PSUM scores):
  * Scalar engine evacuates PSUM -> SBUF converting to fp16 *into the high
    u16 lane* of a packed fp32 word whose low u16 lane is the key index
    (preloaded from a host template).  Numeric fp32 ordering of the packed
    word == lexicographic (fp16 score, index) ordering, so a single DVE
    Max top-8 per 1024-chunk yields values *and* indices in one pass - no
    MaxIndex, no scatter.
  * A per-query bias (-(q.mbar - c0 + 3.9|q|)) is added during evacuation to
    center the interesting (top-16) scores near zero, which shrinks the fp16
    rounding error where it matters.  Any per-query constant preserves the
    within-row order, so this never breaks correctness.
  * L2: Max + MatchReplace + Max over the 64 chunk-candidates -> top-16
    packed words; low lanes are the global key indices.
  * Indices are relayed out to the dma_gather wrap layout (16 partitions,
    idx[n%16, n//16] = nn-index of slot n = k*128+q) with 16 tiny 16x16 PE
    transposes, then one InstDMAGatherAnt fetches all 2048 neighbor rows of
    v^T per query tile.

Sharding: 8 cores; core c handles batch c//4, query block c%4 (2048 queries),
against all 8192 keys of its batch.
"""

import os
import sys

import numpy as np

sys.path.insert(0, "/opt/trn_rl_repo")

import concourse.bacc as bacc
import concourse.bass as bass
import concourse.mybir as mybir
import concourse.tile as tile
from concourse.masks import make_identity

F32 = mybir.dt.float32
I32 = mybir.dt.int32
F16 = mybir.dt.float16
U16 = mybir.dt.uint16
I16 = mybir.dt.int16

BN_EPS = 1e-5
LRELU_SLOPE = 0.2
BIAS_KAPPA = 3.9


def default_cfg():
    return dict(C=64, NK=8192, NQ=2048, K=16, CH=1024, B_DIST=1,
                TAIL_DIST=4)


def emit(tc, ins, outs, cfg, uniq=""):
    """Per-core program.

    ins:  f (64, NK) f16, fq (64, NQ) f16 (this core's query slice),
          w2t (C, C) f16, wat (C, C) f16,
          mb (128, 1) f16  [rows 0:64 = -mean_keys(f), rows 64:128 = c0/64],
          pkt (128, NK) f32 [u32 words: low u16 = column index, high = 0]
    outs: out_o (NQ, C) f16   max-over-K of u+v (pre-BN), query-major
          out_s (128, 512) f32  rows {0,32,64,96} = psum stats
    """
    nc = tc.nc
    C = cfg["C"]          # 64
    NK = cfg["NK"]        # 8192
    NQ = cfg["NQ"]        # 2048
    K = cfg["K"]          # 16
    CH = cfg["CH"]        # 1024  L1 top-8 chunk
    DB = cfg.get("B_DIST", 2)     # stageB(t-DB): idx relay + gather
    TD = cfg.get("TAIL_DIST", 4)  # tail(t-TD): A=G+u, stats, max, out
    # Every cross-engine dependency is >= 1 full iteration stale, so each
    # in-order engine queue runs back-to-back at its own pace and the
    # period is set by the busiest engine (DVE), not by the
    # Max->L2->transpose->gt->gather->add->stats relay chain.
    NQT = NQ // 128       # 16 query tiles
    NCH = NK // CH        # 8 chunks
    CAND = 8 * NCH        # 64 candidates
    H = K * C // 2        # 512 (stats half-width)

    f, fq, w2t, wat, mb = (ins["f"], ins["fq"], ins["w2t"], ins["wat"],
                           ins["mb"])
    out_o, out_s = outs["out_o"], outs["out_s"]

    from contextlib import ExitStack
    ctx = ExitStack()
    dram_pool = ctx.enter_context(tc.tile_pool(name="dram" + uniq, bufs=1,
                                               space="DRAM"))
    vt_dram = dram_pool.tile([NK, 2 * C], F16, tag="vt", name="vt_t" + uniq)[:]

    cpool = ctx.enter_context(tc.tile_pool(name="consts" + uniq, bufs=1))
    big = ctx.enter_context(tc.tile_pool(name="big" + uniq, bufs=1))
    tk = ctx.enter_context(tc.tile_pool(name="topk" + uniq, bufs=3))
    vpool = ctx.enter_context(tc.tile_pool(name="vstage" + uniq, bufs=8))
    gpool = ctx.enter_context(tc.tile_pool(name="gather" + uniq, bufs=4))
    tpool = ctx.enter_context(tc.tile_pool(name="tail" + uniq, bufs=4))
    ps_score = ctx.enter_context(tc.tile_pool(name="ps_score" + uniq, bufs=2,
                                              space="PSUM"))
    ps_stat = ctx.enter_context(tc.tile_pool(name="ps_stat" + uniq, bufs=1,
                                             space="PSUM"))
    ps_misc = ctx.enter_context(tc.tile_pool(name="ps_misc" + uniq, bufs=3,
                                             space="PSUM"))

    # ---- constants / inputs ----
    identity = cpool.tile([128, 128], F32, tag="ident")
    make_identity(nc, identity[:])
    ones64 = cpool.tile([64, 1], F16, tag="ones64")
    nc.vector.memset(ones64[:], 1.0)
    ones128 = cpool.tile([128, 1], F16, tag="ones128")
    nc.vector.memset(ones128[:], 1.0)
    w2t_sb = cpool.tile([128, C], F16, tag="w2t")
    wat_sb = cpool.tile([C, C], F16, tag="wat")
    mb_sb = cpool.tile([128, 1], F16, tag="mb")

    # keys_aug: rows 0:64 = f, rows 64:128 = -f^2/2 (built in place from a
    # second copy of f so every engine op stays partition-aligned).
    # DMAs are spread across engine queues so the prologue isn't serialized
    # on SP.  Prologue engine placement keeps the steady-state bottleneck
    # (DVE) light: f^2 squaring runs on DVE (idle in prologue, 2-byte 2x
    # path), v^T/u^T PSUM evacuation on Act.
    keys_aug = big.tile([128, NK], F16, tag="keys_aug")
    q_aug = big.tile([128, NQ], F16, tag="q_aug")
    NQR = NK // 4
    # raw f lands in a scratch tile on partitions 64:128; f^2 writes
    # keys_aug[64:128] lane-aligned from it.  This takes the vt matmuls
    # (which also read fraw) off the first-score critical chain.
    fraw = big.tile([128, NK], F16, tag="fraw")
    for r in range(4):
        nc.scalar.dma_start(out=fraw[64:128, bass.ts(r, NQR)],
                            in_=f[:, bass.ts(r, NQR)])
    nc.sync.dma_start(out=q_aug[0:64, :], in_=fq)
    # w2t lives at base partition 64 to pair with keys_aug[64:128] in matmuls
    nc.sync.dma_start(out=w2t_sb[64:128, :], in_=w2t)
    nc.sync.dma_start(out=wat_sb[:], in_=wat)
    nc.sync.dma_start(out=mb_sb[:], in_=mb)
    # rows 64:128 hold -0.5 so the score matmul contracts -0.5 * f^2 rows
    nc.gpsimd.memset(q_aug[64:128, :], -0.5)

    # fq^2 for the per-query |q| bias estimate (DVE, 2-byte fast path)
    q2 = big.tile([64, NQ], F16, tag="q2")
    nc.vector.tensor_tensor(out=q2[:], in0=q_aug[0:64, :], in1=q_aug[0:64, :],
                            op=mybir.AluOpType.mult)

    # Single packed score buffer (low u16 lanes = key index template).  The
    # evac(t+1, c) -> Max(t, c) WAR is 8 chunk-slots stale, so one buffer
    # pipelines with no stalls.  keys1 quarters and template chunks
    # interleave on SP so chunk c's template lands just before Max(0, c).
    packed2 = [big.tile([128, NK], F32, tag=f"packed{i}",
                        name=f"packed{i}" + uniq) for i in range(2)]
    for r in range(4):
        nc.sync.dma_start(out=keys_aug[0:64, bass.ts(r, NQR)],
                          in_=f[:, bass.ts(r, NQR)])
    # gather index tiles (rows 16:128 must stay 0 for the executor's
    # bounds check; only rows 0:32 are consumed).  Two buffers so tile t's
    # index relay never waits for tile t-1's gathers to finish reading.
    gtidx2 = []
    for gi in range(2):
        gx = big.tile([128, 128], I16, tag=f"gtidx{gi}")
        nc.gpsimd.memset(gx[:], 0)
        gtidx2.append(gx)
    # index template: generated on-chip by Pool iota (u32 word = column
    # index; high u16 stays 0 for idx < 2^16), no HBM traffic at all.
    # packed0 chunk c is needed by Max(0, c); packed1 chunk c by evac(1, c).
    def emit_tpl(buf, c):
        nc.gpsimd.iota(packed2[buf][:, c * CH:(c + 1) * CH].bitcast(I32),
                       pattern=[[1, CH]], base=c * CH, channel_multiplier=0)

    for c in range(4):
        emit_tpl(0, c)

    # f^2 quarters are emitted inside stageA(0) (prologue_chunk_hook)
    # just before the chunks that need them; the v^T build is emitted after
    # stageA(0) so the first tile's score pipeline owns the engine queues.
    def emit_fsq(r):
        nc.vector.tensor_tensor(
            out=keys_aug[64:128, bass.ts(r, NQR)],
            in0=fraw[64:128, bass.ts(r, NQR)],
            in1=fraw[64:128, bass.ts(r, NQR)],
            op=mybir.AluOpType.mult)

    def prologue_chunk_hook(c):
        if c % 2 == 0:
            emit_fsq(c // 2)
        if c + 4 < NCH:
            emit_tpl(0, c + 4)
        emit_tpl(1, c)
        # one v^T block per chunk: PE stays warm between score matmuls and
        # the whole table is written by the end of tile 0 (gathers start at
        # iteration 2); evac copies alternate Act/DVE to share the load
        emit_vt([c], alt=c % 2)

    vt_view = vt_dram.rearrange("(t c p) ch -> t p c ch", t=8, c=8)

    def emit_vt(ts, alt=0):
        for t in ts:
            ps_vt = ps_misc.tile([128, 512], F32, tag="misc")
            for c in range(8):
                nc.tensor.matmul(ps_vt[:, c * C:(c + 1) * C],
                                 lhsT=fraw[64:128, bass.ts(8 * t + c, 128)],
                                 rhs=w2t_sb[64:128, :], start=True, stop=True)
            vt_sb = vpool.tile([128, 512], F16, tag="vt_sb")
            if alt:
                nc.vector.tensor_copy(out=vt_sb[:], in_=ps_vt[:])
            else:
                nc.scalar.copy(vt_sb[:], ps_vt[:])
            nc.sync.dma_start(out=vt_view[t][:, :, 0:C],
                              in_=vt_sb[:].rearrange("p (c ch) -> p c ch", c=8))
            nc.sync.dma_start(out=vt_view[t][:, :, C:2 * C],
                              in_=vt_sb[:].rearrange("p (c ch) -> p c ch", c=8))

    # per-query bias for all tiles: bias_all[:, t] = -(q.mbar - c0 + kappa*|q|)
    ps_ball = ps_misc.tile([128, 512], F32, tag="misc", name="psball" + uniq)
    for qt in range(NQT):
        nc.tensor.matmul(ps_ball[:, qt:qt + 1], lhsT=q_aug[:, bass.ts(qt, 128)],
                         rhs=mb_sb[:], start=True, stop=True)
        nc.tensor.matmul(ps_ball[:, NQT + qt:NQT + qt + 1],
                         lhsT=q2[:, bass.ts(qt, 128)], rhs=ones64[:],
                         start=True, stop=True)
    qn_all = big.tile([128, NQT], F32, tag="qn_all")
    nc.scalar.activation(qn_all[:], ps_ball[:, NQT:2 * NQT],
                         mybir.ActivationFunctionType.Sqrt, scale=1.0)
    bias_all = big.tile([128, NQT], F32, tag="bias_all")
    nc.vector.scalar_tensor_tensor(
        out=bias_all[:], in0=qn_all[:], scalar=-BIAS_KAPPA,
        in1=ps_ball[:, 0:NQT],
        op0=mybir.AluOpType.mult, op1=mybir.AluOpType.add)

    ut_sb = big.tile([128, NQT * C], F16, tag="ut")
    for h in range(2):
        ps_ut = ps_misc.tile([128, 512], F32, tag="misc")
        for j in range(8):
            nc.tensor.matmul(ps_ut[:, j * C:(j + 1) * C],
                             lhsT=q_aug[0:64, bass.ts(8 * h + j, 128)],
                             rhs=wat_sb[:], start=True, stop=True)
        nc.scalar.copy(ut_sb[:, h * 512:(h + 1) * 512], ps_ut[:])

    stats_ps = ps_stat.tile([128, 512], F32, tag="stat")

    # ---- per-tile tail, split so every engine's inputs are a full
    # iteration old when its queue reaches them ----
    # tailP(t): Pool A = G + u, A2 = A^2 (emitted FIRST in the iteration so
    #           they precede the gathers in Pool's in-order queue)
    # tailC(t): PE stats matmuls + DVE max-over-K fold + out DMA (reads
    #           A/A2 produced one iteration earlier)
    def tailP(tq):
        G = G_tiles.pop(tq)
        ut_qt = ut_sb[:, bass.ts(tq, C)]
        A = tpool.tile([128, K * C], F16, tag="A")
        nc.vector.tensor_tensor(
            out=A[:].rearrange("p (k o) -> p k o", k=K),
            in0=G[:].rearrange("p (k o) -> p k o", k=K)[:, :, 0:C],
            in1=ut_qt.unsqueeze(1).broadcast_to([128, K, C]),
            op=mybir.AluOpType.add)
        A_tiles[tq] = A

    def tailQ(tq):
        A = A_tiles[tq]
        A2 = tpool.tile([128, K * C], F16, tag="A2")
        # Square on Act (same act-table set as Identity/Copy/Sqrt, so no
        # table reloads); keeps Pool at gathers only.  Last tiles land in
        # the drain where DVE is idle - run those there.
        if tq >= NQT - 4:
            nc.vector.tensor_tensor(out=A2[:], in0=A[:], in1=A[:],
                                    op=mybir.AluOpType.mult)
        else:
            nc.scalar.activation(A2[:], A[:],
                                 mybir.ActivationFunctionType.Square,
                                 bias=0.0, scale=1.0)
        A2_tiles[tq] = A2

    def tailC(tq):
        A = A_tiles.pop(tq)
        A2 = A2_tiles.pop(tq)
        st, sp = (tq == 0), (tq == NQT - 1)
        nc.tensor.matmul(stats_ps[0:1, 0:H], lhsT=ones128[:], rhs=A[:, 0:H],
                         start=st, stop=sp, tile_position=(0, 0))
        nc.tensor.matmul(stats_ps[32:33, 0:H], lhsT=ones128[:], rhs=A[:, H:],
                         start=st, stop=sp, tile_position=(0, 32))
        nc.tensor.matmul(stats_ps[64:65, 0:H], lhsT=ones128[:], rhs=A2[:, 0:H],
                         start=st, stop=sp, tile_position=(0, 64))
        nc.tensor.matmul(stats_ps[96:97, 0:H], lhsT=ones128[:], rhs=A2[:, H:],
                         start=st, stop=sp, tile_position=(0, 96))
        # max over K: fold tree (8,4,2,1) on DVE fp16 (2x path; Pool has no
        # ISA max, and contiguous halves keep the 2-byte fast mode)
        M1 = tpool.tile([128, 8 * C], F16, tag="M1")
        nc.vector.tensor_tensor(out=M1[:], in0=A[:, 0:8 * C], in1=A[:, 8 * C:],
                                op=mybir.AluOpType.max)
        M2 = tpool.tile([128, 4 * C], F16, tag="M2")
        nc.vector.tensor_tensor(out=M2[:], in0=M1[:, 0:4 * C], in1=M1[:, 4 * C:],
                                op=mybir.AluOpType.max)
        M3 = tpool.tile([128, 2 * C], F16, tag="M3")
        nc.vector.tensor_tensor(out=M3[:], in0=M2[:, 0:2 * C], in1=M2[:, 2 * C:],
                                op=mybir.AluOpType.max)
        O = tpool.tile([128, C], F16, tag="O")
        nc.vector.tensor_tensor(out=O[:], in0=M3[:, 0:C], in1=M3[:, C:],
                                op=mybir.AluOpType.max)
        # out DMA rides SP, whose queue has nothing else in steady state,
        # so its wait on O can't block any other work.
        nc.sync.dma_start(out=out_o[bass.ts(tq, 128), :], in_=O[:])

    # ---- main loop over query tiles (software pipelined) ----
    # stage A(t): scores -> packed -> L1/L2 top-16 -> idxf
    # stage B(t): idx relayout (PE transposes) -> gtidx -> dma_gather
    G_tiles = {}
    idx_tiles = {}
    Xsb_tiles = {}
    A_tiles = {}
    A2_tiles = {}

    def stageA(qt):
        qcols = bass.ts(qt, 128)
        pk = packed2[qt % 2]
        pk_hi = pk[:].bitcast(F16).rearrange("p (n two) -> p n two", two=2)

        # scores -> packed fp16-high-lane (+bias) -> L1 top-8 per chunk
        Ct = tk.tile([128, CAND], F32, tag="C")
        for c in range(NCH):
            if qt == 0:
                prologue_chunk_hook(c)
            ps_s = ps_score.tile([128, CH], F32, tag="score",
                                 name=f"pss{uniq}_{qt}_{c}")
            nc.tensor.matmul(ps_s[:, 0:512], lhsT=q_aug[:, qcols],
                             rhs=keys_aug[:, c * CH:c * CH + 512],
                             start=True, stop=True)
            nc.tensor.matmul(ps_s[:, 512:1024], lhsT=q_aug[:, qcols],
                             rhs=keys_aug[:, c * CH + 512:(c + 1) * CH],
                             start=True, stop=True)
            nc.scalar.activation(pk_hi[:, c * CH:(c + 1) * CH, 1:2], ps_s[:],
                                 mybir.ActivationFunctionType.Identity,
                                 bias=bias_all[:, qt:qt + 1], scale=1.0)
            nc.vector.max(out=Ct[:, bass.ts(c, 8)],
                          in_=pk[:, c * CH:(c + 1) * CH])


        # L2: top-16 packed words
        T12 = tk.tile([128, K], F32, tag="T12")
        Cmr = tk.tile([128, CAND], F32, tag="Cmr")
        nc.vector.max(out=T12[:, 0:8], in_=Ct[:])
        nc.vector.match_replace(out=Cmr[:], in_to_replace=T12[:, 0:8],
                                in_values=Ct[:], imm_value=-1e30)
        nc.vector.max(out=T12[:, 8:16], in_=Cmr[:])
        idxf = tk.tile([128, K], F32, tag="idxf")
        nc.vector.tensor_copy(
            out=idxf[:].unsqueeze(2),
            in_=T12[:].bitcast(U16).rearrange("p (n two) -> p n two",
                                              two=2)[:, :, 0:1])
        idx_tiles[qt] = idxf

    def stageB1(qt):
        idxf = idx_tiles.pop(qt)
        # relayout idx [128q, 16k] -> wrap layout [16, 128]:
        # X[k, q] = idx[q, k] (one full transpose).  PE transpose runs after
        # this iteration's score matmuls; the Act copy sits after this
        # iteration's evacs, by which time ps_x is long done.
        ps_x = ps_misc.tile([128, 512], F32, tag="misc", name=f"psx{uniq}_{qt}")
        nc.tensor.transpose(ps_x[0:16, 0:128], idxf[:], identity[:])
        Xsb = tk.tile([16, 128], F32, tag="Xsb")
        nc.scalar.copy(Xsb[:], ps_x[0:16, 0:128])
        Xsb_tiles[qt] = Xsb

    def stageB2(qt):
        Xsb = Xsb_tiles.pop(qt)
        # per-16-block transposes ps_y[b, 16a+k] = X[k, 16a+b] (all
        # base-partition 0).  The block transposes take a free-duplicated
        # input so the output lands twice (partitions 0:16 and 16:32) - the
        # gather's tx Q7 core reads the index rows from partitions 16:32.
        ps_y = ps_misc.tile([128, 512], F32, tag="misc", name=f"psy{uniq}_{qt}")
        for a in range(8):
            nc.tensor.transpose(ps_y[0:16, 16 * a:16 * (a + 1)],
                                Xsb[:, 16 * a:16 * (a + 1)],
                                identity[0:16, 0:16])
        gt = gtidx2[qt % 2]
        # gtidx[b, 8k+a] = ps_y[b, 16a+k]; the gather's tx Q7 core reads the
        # index rows from partitions 16:32, so DMA-replicate them there.
        nc.scalar.activation(
            gt[0:16, :],
            ps_y[0:16, 0:128].rearrange("p (a k) -> p k a", a=8),
            mybir.ActivationFunctionType.Copy)
        # replicate rides the Act queue right after the copy that feeds it
        nc.scalar.dma_start(out=gt[16:32, :], in_=gt[0:16, :])

        # gather neighbor features G[q, k, 0:C] = v^T[nn[q, k], :]
        # (4 ops of 512 idxs - the Q7 idx scratch caps num_idxs at 512;
        #  f16 rows padded to 256 bytes to satisfy the descriptor-size rule)
        G = gpool.tile([128, K * 2 * C], F16, tag="G")
        Gv = G[:].rearrange("p (k o) -> p k o", k=K)
        for g in range(4):
            nc.gpsimd.dma_gather(
                out_ap=Gv[:, 4 * g:4 * (g + 1), :],
                in_ap=vt_dram,
                idxs_ap=gt[:, 32 * g:32 * (g + 1)],
                num_idxs=512,
                num_idxs_reg=512,
                elem_size=2 * C,
            )
        G_tiles[qt] = G

    # Virtual-iteration schedule.  Stage offsets (it = virtual iteration):
    #   stageA(it)     scores/evac/L1+L2 top-k   PE/Act/DVE
    #   stageB1(it-2)  idx transpose + Xsb       PE tail / Act tail
    #   stageB2(it-3)  blocks + gt + gathers     PE / Act tail / Pool tail
    #   tailP(it-4)    A = G+u, A2               Pool head
    #   tailC(it-6)    stats + fold + out        PE head / DVE head / SP
    # All cross-engine inputs are >= 1 iteration old except the intended
    # mm -> evac -> Max chunk pipeline and the same-iteration PE -> Act
    # relay (PE runs those ~4us before Act reaches them).
    SL = cfg.get("stage_limit", 3)
    TC = TD + 2
    for it in range(NQT + TC):
        if SL >= 3 and 0 <= it - TC:
            tailC(it - TC)
        if SL >= 3 and 0 <= it - TD - 1 < NQT:
            tailQ(it - TD - 1)
        if SL >= 3 and 0 <= it - TD < NQT:
            tailP(it - TD)
        late = it >= NQT - 1
        if late and SL >= 2 and 0 <= it - DB < NQT:
            stageB1(it - DB)
        if late and SL >= 2 and 0 <= it - DB - 1 < NQT:
            stageB2(it - DB - 1)
        if it < NQT:
            stageA(it)

        if not late and SL >= 2 and 0 <= it - DB < NQT:
            stageB1(it - DB)
        if not late and SL >= 2 and 0 <= it - DB - 1 < NQT:
            stageB2(it - DB - 1)

    # ---- epilogue: stats psum -> sbuf (4 rows) -> dram ----
    if SL >= 3:
        s_sb = big.tile([128, 512], F32, tag="s_sb")
        for p in (0, 32, 64, 96):
            nc.vector.tensor_copy(out=s_sb[p:p + 1, :],
                                  in_=stats_ps[p:p + 1, :])
        nc.sync.dma_start(
            out=out_s,
            in_=s_sb[:].rearrange("(r s) h -> r s h", s=32)[:, 0:1, :])
    ctx.close()


def build_program(cfg, num_cores=8, reps=1):
    nc = bacc.Bacc("TRN2", target_bir_lowering=False, debug=False,
                   enable_asserts=False, num_devices=num_cores)
    C, NK, NQ = cfg["C"], cfg["NK"], cfg["NQ"]
    ins = {
        "f": nc.dram_tensor("f", [C, NK], F16, kind="ExternalInput").ap(),
        "fq": nc.dram_tensor("fq", [C, NQ], F16, kind="ExternalInput").ap(),
        "w2t": nc.dram_tensor("w2t", [C, C], F16, kind="ExternalInput").ap(),
        "wat": nc.dram_tensor("wat", [C, C], F16, kind="ExternalInput").ap(),
        "mb": nc.dram_tensor("mb", [128, 1], F16, kind="ExternalInput").ap(),
    }
    outs = {
        "out_o": nc.dram_tensor("out_o", [NQ, C], F16,
                                kind="ExternalOutput").ap(),
        "out_s": nc.dram_tensor("out_s", [4, 512], F32,
                                kind="ExternalOutput").ap(),
    }
    with tile.TileContext(nc) as tc:
        for r in range(reps):
            emit(tc, ins, outs, cfg, uniq=f"_r{r}")
    nc.compile()
    return nc


_PROGRAM_CACHE = {}


def get_program(num_cores=8):
    key = num_cores
    if key not in _PROGRAM_CACHE:
        _PROGRAM_CACHE[key] = build_program(default_cfg(), num_cores)
    return _PROGRAM_CACHE[key]


def make_in_maps(x, W, n_cores=8):
    """Build the per-core input dicts from the full inputs."""
    cfg = default_cfg()
    C, NQ = cfg["C"], cfg["NQ"]
    B, _, N, _ = x.shape
    per_batch = N // NQ
    f16 = np.ascontiguousarray(x[:, :, :, 0]).astype(np.float16)  # (B, C, N)
    W16 = W.astype(np.float16)
    W1, W2 = W16[:, :C], W16[:, C:]
    w2t = np.ascontiguousarray(W2.T)
    wat = np.ascontiguousarray((W1 - W2).T)
    mbs = []
    for b in range(B):
        fb = f16[b].astype(np.float32)
        mbar = fb.mean(axis=1)                      # (C,)
        c0 = float(0.5 * (fb * fb).sum(axis=0).mean())
        mbv = np.zeros((128, 1), np.float16)
        mbv[0:C, 0] = (-mbar).astype(np.float16)
        # contracted against the -0.5 rows of q_aug: 64 * (-0.5) * (-c0/32) = c0
        mbv[C:128, 0] = np.float16(-c0 / 32.0)
        mbs.append(mbv)
    in_maps = []
    for c in range(n_cores):
        b, qb = c // per_batch, c % per_batch
        in_maps.append({
            "f": np.ascontiguousarray(f16[b]),
            "fq": np.ascontiguousarray(f16[b][:, qb * NQ:(qb + 1) * NQ]),
            "w2t": w2t,
            "wat": wat,
            "mb": mbs[b],
        })
    return in_maps


def host_epilogue(m_full, s1, s2, gamma, beta, count):
    mean = s1 / count
    var = s2 / count - mean * mean
    a = gamma.astype(np.float64) / np.sqrt(var + BN_EPS)
    b = beta.astype(np.float64) - a * mean
    y = a[None, :, None] * m_full.astype(np.float64) + b[None, :, None]
    y = np.where(y >= 0, y, LRELU_SLOPE * y)
    return y.astype(np.float32)


def kernel(x, W, gamma, beta):
    """Full (unsharded) inputs -> full output. See module docstring."""
    from concourse import bass_utils

    x = np.asarray(x)
    W = np.asarray(W)
    gamma = np.asarray(gamma)
    beta = np.asarray(beta)

    B, C, N, _ = x.shape
    K = 16
    assert (B, C, N) == (2, 64, 8192), "kernel hardcoded for this problem size"

    cfg = default_cfg()
    NQ = cfg["NQ"]
    n_cores = 8
    per_batch = N // NQ

    in_maps = make_in_maps(x, W, n_cores)
    nc = get_program(n_cores)
    res = bass_utils.run_bass_kernel_spmd(nc, in_maps, list(range(n_cores)))
    results = res.results

    m_full = np.empty((B, C, N), np.float32)
    s1 = np.zeros(C, np.float64)
    s2 = np.zeros(C, np.float64)
    H = K * C // 2
    for c in range(n_cores):
        b, qb = c // per_batch, c % per_batch
        m_full[b, :, qb * NQ:(qb + 1) * NQ] = \
            results[c]["out_o"].astype(np.float32).T
        st = results[c]["out_s"].astype(np.float64)
        s1 += (st[0, :H].reshape(K // 2, C) + st[1, :H].reshape(K // 2, C)).sum(0)
        s2 += (st[2, :H].reshape(K // 2, C) + st[3, :H].reshape(K // 2, C)).sum(0)

    count = float(B) * N * K
    return host_epilogue(m_full, s1, s2, gamma, beta, count)


if __name__ == "__main__":
    sys.path.insert(0, os.path.dirname(os.path.abspath(__file__)))
    import reference

    inputs = {k: np.asarray(v) for k, v in reference.setup_inputs().items()}
    out = kernel(**inputs)
    import jax
    cpu = jax.devices("cpu")[0]
    with jax.default_device(cpu):
        exp = np.asarray(reference.reference(
            **{k: jax.device_put(v, cpu) for k, v in inputs.items()}))
    err = np.abs(out - exp)
    rel = np.linalg.norm(out - exp) / np.linalg.norm(exp)
    print("max abs err:", err.max(), "rel l2 err:", rel)

